# revision 27
# baseline (speedup 1.0000x reference)
"""BERT-NER (12-layer BERT-base + token compaction + classifier) on 8 TRN2 cores.

Data-parallel over batch: 16 sequences -> 2 per core. Weights replicated.
Activations are kept feature-major (xT: [768 partitions(6 tiles), 512 tokens])
so every x@W matmul uses the stored W[in,out] directly as lhsT.
All matmuls run in float32r (full PE rate for N>=256), storage stays fp32.
LayerNorm stats, softmax row-sums and free-dim biases are folded into small
matmuls (ones vectors) to avoid partition-dim reductions on DVE.
"""

import os
import sys

for _p in ("/opt/trn_rl_repo", "/root/.axon_site/_ro/trn_rl_repo"):
    if os.path.isdir(_p) and _p not in sys.path:
        sys.path.insert(0, _p)

import numpy as np

import concourse.bass as bass
import concourse.mybir as mybir
import concourse.tile as tile
from concourse.tile import add_dep_helper
from concourse import bacc, bass_utils

F32 = mybir.dt.float32
F32R = mybir.dt.float32r
I32 = mybir.dt.int32
AF = mybir.ActivationFunctionType
ALU = mybir.AluOpType

B, S, H, L, A, V, NL = 16, 256, 768, 12, 12, 30522, 9
DH = H // A  # 64
FF = 4 * H  # 3072
NC = 8  # cores
BL = B // NC  # 2 sequences per core
T = BL * S  # 512 tokens per core
KT = H // 128  # 6 k-tiles of the hidden dim
TC = T // 128  # 4 token chunks
BIG = 1_000_000  # OOB dump index for compaction scatter
EPS = 1e-12
ISCALE = 1.0 / float(np.sqrt(DH))

P = 128


def _r(ap):
    """View an AP as float32r (bit-identical, PE reduced precision)."""
    return ap.bitcast(F32R)


def _f(ap):
    """View an f32r AP as plain float32 for DVE/ACT reads."""
    return ap.bitcast(F32)


def build_nc(repeat=1, n_layers=L):
    nc = bacc.Bacc("TRN2", target_bir_lowering=False, debug=False)

    d_ids = nc.dram_tensor("input_word_ids", [BL, S], I32, kind="ExternalInput")
    d_mask = nc.dram_tensor("input_mask", [BL, S], I32, kind="ExternalInput")
    d_type = nc.dram_tensor("input_type_ids", [BL, S], I32, kind="ExternalInput")
    d_valid = nc.dram_tensor("valid_mask", [BL, S], I32, kind="ExternalInput")
    d_wemb = nc.dram_tensor("word_emb", [V, H], F32, kind="ExternalInput")
    d_pemb = nc.dram_tensor("pos_emb", [S, H], F32, kind="ExternalInput")
    d_temb = nc.dram_tensor("type_emb", [2, H], F32, kind="ExternalInput")
    d_elng = nc.dram_tensor("emb_ln_g", [H], F32, kind="ExternalInput")
    d_elnb = nc.dram_tensor("emb_ln_b", [H], F32, kind="ExternalInput")
    d_Wq = nc.dram_tensor("Wq", [L, H, H], F32, kind="ExternalInput")
    d_bq = nc.dram_tensor("bq", [L, H], F32, kind="ExternalInput")
    d_Wk = nc.dram_tensor("Wk", [L, H, H], F32, kind="ExternalInput")
    d_bk = nc.dram_tensor("bk", [L, H], F32, kind="ExternalInput")
    d_Wv = nc.dram_tensor("Wv", [L, H, H], F32, kind="ExternalInput")
    d_bv = nc.dram_tensor("bv", [L, H], F32, kind="ExternalInput")
    d_Wo = nc.dram_tensor("Wo", [L, H, H], F32, kind="ExternalInput")
    d_bo = nc.dram_tensor("bo", [L, H], F32, kind="ExternalInput")
    d_alg = nc.dram_tensor("attn_ln_g", [L, H], F32, kind="ExternalInput")
    d_alb = nc.dram_tensor("attn_ln_b", [L, H], F32, kind="ExternalInput")
    d_W1 = nc.dram_tensor("W1", [L, H, FF], F32, kind="ExternalInput")
    d_b1 = nc.dram_tensor("b1", [L, FF], F32, kind="ExternalInput")
    d_W2 = nc.dram_tensor("W2", [L, FF, H], F32, kind="ExternalInput")
    d_b2 = nc.dram_tensor("b2", [L, H], F32, kind="ExternalInput")
    d_flg = nc.dram_tensor("ffn_ln_g", [L, H], F32, kind="ExternalInput")
    d_flb = nc.dram_tensor("ffn_ln_b", [L, H], F32, kind="ExternalInput")
    d_clsW = nc.dram_tensor("cls_W", [H, NL], F32, kind="ExternalInput")
    d_clsb = nc.dram_tensor("cls_b", [NL], F32, kind="ExternalInput")
    d_out = nc.dram_tensor("out", [BL, S, NL], F32, kind="ExternalOutput")

    dr = dict(
        ids=d_ids, mask=d_mask, type=d_type, valid=d_valid, wemb=d_wemb,
        pemb=d_pemb, temb=d_temb, elng=d_elng, elnb=d_elnb,
        Wq=d_Wq, bq=d_bq, Wk=d_Wk, bk=d_bk, Wv=d_Wv, bv=d_bv, Wo=d_Wo, bo=d_bo,
        alg=d_alg, alb=d_alb, W1=d_W1, b1=d_b1, W2=d_W2, b2=d_b2,
        flg=d_flg, flb=d_flb, clsW=d_clsW, clsb=d_clsb, out=d_out,
    )

    with nc.allow_low_precision(reason="float32r matmul pipeline"), tile.TileContext(
        nc
    ) as tc:
        with (
            tc.tile_pool(name="const", bufs=1) as cpool,
            tc.tile_pool(name="main", bufs=1) as mpool,
            tc.tile_pool(name="wts", bufs=3) as wpool,
            tc.tile_pool(name="w2p", bufs=3) as w2pool,
            tc.tile_pool(name="hrows", bufs=2) as rpool,
            tc.tile_pool(name="hbuf", bufs=3) as hpool,
            tc.tile_pool(name="ebuf", bufs=4) as epool,
            tc.tile_pool(name="small", bufs=2) as spool,
        ):
            pools = dict(c=cpool, m=mpool, w=wpool, w2=w2pool, r=rpool,
                         h=hpool, e=epool, s=spool)
            # ---- constants (device-generated) ----
            ident = cpool.tile([P, P], F32, tag="ident")
            nc.gpsimd.memset(ident[:], 0.0)
            nc.gpsimd.affine_select(
                out=ident[:], in_=ident[:], compare_op=ALU.not_equal, fill=1.0,
                base=0, pattern=[[-1, P]], channel_multiplier=1,
            )
            ones_f32 = cpool.tile([P, 512], F32, tag="ones_f32")
            nc.gpsimd.memset(ones_f32[:], 1.0)
            ones_row = cpool.tile([1, 512], F32R, tag="ones_row")
            nc.vector.tensor_copy(out=ones_row[:], in_=ones_f32[:1, :])
            ones128 = cpool.tile([P, P], F32R, tag="ones128")
            nc.vector.tensor_copy(out=ones128[:], in_=ones_f32[:, :P])
            ones256r = cpool.tile([P, S], F32R, tag="ones256r")
            nc.vector.tensor_copy(out=ones256r[:], in_=ones_f32[:, :S])
            # onehot12[:, h, :]: [128, 12] lhsT whose column h is all-ones --
            # rowsum matmul lands head h's softmax denominator on partition h.
            # (built in f32, then DVE-copied to f32r: fp32r matmul operands
            # must come from a rounding producer, the BIR verifier checks.)
            onehot12f = cpool.tile([P, A, A], F32, tag="cscratch")
            nc.gpsimd.memset(onehot12f[:], 1.0)
            nc.gpsimd.affine_select(
                out=onehot12f[:], in_=onehot12f[:], compare_op=ALU.is_equal,
                fill=0.0, base=0, pattern=[[-1, A], [1, A]],
                channel_multiplier=0,
            )
            onehot12 = cpool.tile([P, A, A], F32R, tag="onehot12")
            nc.vector.tensor_copy(out=onehot12[:], in_=onehot12f[:])
            zero12f = cpool.tile([P, A], F32, tag="cscratch")
            nc.gpsimd.memset(zero12f[:], 0.0)
            zero12 = cpool.tile([P, A], F32R, tag="zero12")
            nc.vector.tensor_copy(out=zero12[:], in_=zero12f[:])
            # dsel[:, ft]: [12, 128] lhsT; column m is one-hot at row
            # 2*ft + (m >= 64) -- broadcasts the two per-head denominator rows
            # of tile ft across its 2x64 partitions.
            dself = cpool.tile([A, KT, 2, DH], F32, tag="cscratch")
            nc.gpsimd.memset(dself[:], 1.0)
            nc.gpsimd.affine_select(
                out=dself[:], in_=dself[:], compare_op=ALU.is_equal, fill=0.0,
                base=0, pattern=[[-2, KT], [-1, 2], [0, DH]],
                channel_multiplier=1,
            )
            dsel = cpool.tile([A, KT, 2, DH], F32R, tag="dsel")
            nc.vector.tensor_copy(out=dsel[:], in_=dself[:])
            # lower-triangular-inclusive block: tri[p, t] = 1 if p <= t
            ltri_f = cpool.tile([P, P], F32, tag="ltri_f")
            nc.gpsimd.memset(ltri_f[:], 1.0)
            nc.gpsimd.affine_select(
                out=ltri_f[:], in_=ltri_f[:], compare_op=ALU.is_ge, fill=0.0,
                base=0, pattern=[[1, P]], channel_multiplier=-1,
            )
            c_eps = cpool.tile([P, 1], F32, tag="c_eps")
            nc.gpsimd.memset(c_eps[:], EPS)
            consts = dict(ident=ident, ones_row=ones_row, ltri=ltri_f,
                          c_eps=c_eps, ones_f32=ones_f32, ones128=ones128,
                          ones256r=ones256r, onehot12=onehot12,
                          zero12=zero12, dsel=dsel)

            def body():
                emit_body(nc, tc, pools, consts, dr, n_layers)

            if repeat == 1:
                body()
            else:
                with tc.For_i(0, repeat, 1):
                    body()

    nc.compile()
    return nc


def _load_w_full(nc, wpool, d_slice):
    """Load a [H, 768] DRAM slice as SBUF [128, KT, 768] (k-tiles on
    partitions). Split across BOTH HWDGE engines (SP + Activation) -- a
    single engine's queue bottlenecks at ~53 GB/s, both reach ~380 GB/s."""
    w = wpool.tile([P, KT, H], F32R, tag="w_big", name="w_big")
    src = d_slice.rearrange("(kt p) c -> p kt c", p=P)
    nc.sync.dma_start(w[:, 0:3], _r(src[:, 0:3]))
    nc.scalar.dma_start(w[:, 3:6], _r(src[:, 3:6]))
    return w


def _bias_col(nc, spool, d_vec, tag):
    """Load [H] DRAM vector as [128, KT] (col m = slice m*128:(m+1)*128)."""
    t = spool.tile([P, KT], F32, tag=tag, name=tag)
    nc.sync.dma_start(t[:], d_vec.rearrange("(kt p) -> p kt", p=P))
    return t


def _bias_row(nc, rpool, d_vec, tag="brow", dtype=F32R):
    """Load a DRAM vector [N<=768] as a single-partition row [1, N]."""
    n = d_vec.shape[0]
    t = rpool.tile([1, n], dtype, tag=tag, name=tag)
    nc.sync.dma_start(t[:], d_vec[None, :].bitcast(dtype))
    return t


def emit_ln_half(nc, ppool, mpool, spool, y, hb, g_col, b_col, out, consts,
                 ltag="pp"):
    """Feature-major layernorm over token half hb (columns hb*S..hb*S+S).
    Stat matmuls use an all-ones [128,128] lhsT so per-token sums land
    broadcast across 128 partitions; sum and sum-of-squares share one PSUM
    bank (two regions), ordered by an explicit dep after the bank clear."""
    ones128 = consts["ones128"]
    o = hb * S
    ln = ppool.tile([P, 2, S], F32, tag=ltag, bufs=2, space="PSUM",
                    name="ln_ps")
    mm_clear = None
    for kt in range(KT):
        sq = mpool.tile([P, S], F32R, tag="ln_sq", bufs=2, name="sq")
        nc.scalar.activation(sq[:], _f(y[:, kt, o : o + S]), AF.Square)
        mm1 = nc.tensor.matmul(ln[:, 0], ones128[:], y[:, kt, o : o + S],
                               start=(kt == 0), stop=(kt == KT - 1),
                               skip_group_check=True)
        mm2 = nc.tensor.matmul(ln[:, 1], ones128[:], sq[:],
                               start=False, stop=(kt == KT - 1),
                               skip_group_check=True)
        if kt == 0:
            add_dep_helper(mm2.ins, mm1.ins, reason="sumsq after bank clear")
    mean = spool.tile([P, S], F32, tag="ln_mean", bufs=2, name="ln_mean")
    nc.vector.tensor_scalar_mul(mean[:], ln[:, 0], 1.0 / H)
    m2 = spool.tile([P, S], F32, tag="ln_m2", bufs=1, name="ln_m2")
    nc.vector.tensor_tensor(out=m2[:], in0=mean[:], in1=mean[:], op=ALU.mult)
    var = spool.tile([P, S], F32, tag="ln_var", bufs=1, name="ln_var")
    nc.vector.scalar_tensor_tensor(out=var[:], in0=ln[:, 1], scalar=1.0 / H,
                                   in1=m2[:], op0=ALU.mult, op1=ALU.subtract)
    std = spool.tile([P, S], F32, tag="ln_std", bufs=1, name="ln_std")
    nc.scalar.activation(std[:], var[:], AF.Sqrt, bias=consts["c_eps"][:])
    rstd = spool.tile([P, S], F32, tag="ln_rstd", bufs=2, name="ln_rstd")
    nc.vector.reciprocal(rstd[:], std[:])
    for kt in range(KT):
        tmp = mpool.tile([P, S], F32, tag="ln_tmp", bufs=2, name="tmp")
        nc.vector.tensor_tensor(out=tmp[:], in0=_f(y[:, kt, o : o + S]),
                                in1=mean[:], op=ALU.subtract)
        nc.vector.tensor_tensor(out=tmp[:], in0=tmp[:], in1=rstd[:],
                                op=ALU.mult)
        nc.scalar.activation(out[:, kt, o : o + S], tmp[:], AF.Identity,
                             scale=g_col[:, kt : kt + 1],
                             bias=b_col[:, kt : kt + 1])


def emit_body(nc, tc, pools, consts, dr, n_layers):
    cpool, mpool, wpool, w2pool = (
        pools["c"], pools["m"], pools["w"], pools["w2"])
    rpool, hpool, epool, spool = (
        pools["r"], pools["h"], pools["e"], pools["s"])
    ident, ones_row, ltri = (
        consts["ident"], consts["ones_row"], consts["ltri"])
    ones_f32 = consts["ones_f32"]
    onehot12, zero12, dsel = (
        consts["onehot12"], consts["zero12"], consts["dsel"])
    ones256r = consts["ones256r"]

    ids_flat = dr["ids"].rearrange("b s -> (b s)")
    type_flat = dr["type"].rearrange("b s -> (b s)")
    mask_flat = dr["mask"].rearrange("b s -> (b s)")
    valid_flat = dr["valid"].rearrange("b s -> (b s)")

    # amask[:, c]: 0 where mask==1 else -10000 ; valid_f: valid mask as f32r
    amask = cpool.tile([P, TC], F32, tag="amask", name="amask")
    valid_f = cpool.tile([P, TC], F32, tag="valid_f", name="valid_f")

    # ============ embeddings (token-major), transpose, LN ============
    xtok = mpool.tile([P, TC, H], F32, tag="bigA", name="xtok")
    for c in range(TC):
        idt = spool.tile([P, 1], I32, tag="idt", name="idt")
        nc.sync.dma_start(idt[:], ids_flat[c * P : (c + 1) * P, None])
        nc.gpsimd.indirect_dma_start(
            out=xtok[:, c], out_offset=None, in_=dr["wemb"][:, :],
            in_offset=bass.IndirectOffsetOnAxis(ap=idt[:, :1], axis=0),
        )
        tyt = spool.tile([P, 1], I32, tag="tyt", name="tyt")
        nc.sync.dma_start(tyt[:], type_flat[c * P : (c + 1) * P, None])
        temb = hpool.tile([P, H], F32, tag="embt", bufs=1, name="temb")
        nc.gpsimd.indirect_dma_start(
            out=temb[:], out_offset=None, in_=dr["temb"][:, :],
            in_offset=bass.IndirectOffsetOnAxis(ap=tyt[:, :1], axis=0),
        )
        pemb = hpool.tile([P, H], F32, tag="embt", bufs=1, name="pemb")
        cc = c % (S // P)
        nc.sync.dma_start(pemb[:], dr["pemb"][cc * P : (cc + 1) * P, :])
        nc.vector.tensor_tensor(out=xtok[:, c], in0=xtok[:, c], in1=temb[:],
                                op=ALU.add)
        nc.vector.tensor_tensor(out=xtok[:, c], in0=xtok[:, c], in1=pemb[:],
                                op=ALU.add)

        mi = spool.tile([P, 1], I32, tag="mi", name="mi")
        nc.sync.dma_start(mi[:], mask_flat[c * P : (c + 1) * P, None])
        mf = spool.tile([P, 1], F32, tag="mf", name="mf")
        nc.vector.tensor_copy(out=mf[:], in_=mi[:])
        nc.scalar.activation(amask[:, c : c + 1], mf[:], AF.Copy,
                             scale=10000.0, bias=-10000.0)
        vi = spool.tile([P, 1], I32, tag="vi", name="vi")
        nc.sync.dma_start(vi[:], valid_flat[c * P : (c + 1) * P, None])
        nc.vector.tensor_copy(out=valid_f[:, c : c + 1], in_=vi[:])

    # transpose to feature-major
    xe = mpool.tile([P, KT, 512], F32R, tag="bigB", name="xe")
    x = mpool.tile([P, KT, 512], F32R, tag="x_cur", name="x0")
    with tc.tile_pool(name="embtr", bufs=1, space="PSUM") as ppool:
        for kt in range(KT):
            for c in range(TC):
                ps_t = ppool.tile([P, P], F32, tag="tr", bufs=2, space="PSUM")
                nc.tensor.transpose(
                    out=ps_t[:], in_=xtok[:, c, kt * P : (kt + 1) * P],
                    identity=ident[:])
                nc.vector.tensor_copy(out=xe[:, kt, c * P : (c + 1) * P],
                                      in_=ps_t[:])
        eg = _bias_col(nc, spool, dr["elng"][:], "eg")
        eb = _bias_col(nc, spool, dr["elnb"][:], "eb")
        for hb in range(BL):
            emit_ln_half(nc, ppool, mpool, spool, xe, hb, eg, eb, x, consts)

    # ============ transformer layers ============
    for l in range(n_layers):
        # ---- Q, K projections (feature-major out) ----
        qT = mpool.tile([P, KT, 512], F32R, tag="bigA", name="qT")
        kT = mpool.tile([P, KT, 512], F32R, tag="bigB", name="kT")
        vsb = mpool.tile([P, TC, A, DH], F32R, tag="vsb", name="vsb")
        craw = mpool.tile([P, KT, 512], F32R, tag="bigC", name="craw")
        ctxT = mpool.tile([P, KT, 512], F32R, tag="bigD", name="ctxT")
        y1 = mpool.tile([P, KT, 512], F32R, tag="bigE", name="y1")
        x2 = mpool.tile([P, KT, 512], F32R, tag="bigF", name="x2")

        # ---- phase 1: QKV projections + attention + Wo + LN1, pipelined by
        # sequence half (the two sequences are independent until the FFN's
        # shared weight streaming). All matmuls are N=256 (full f32r rate).
        with tc.tile_pool(name=f"ph1_{l}", bufs=1, space="PSUM") as ppool:
            bq_c = _bias_col(nc, spool, dr["bq"][l], "b_q")
            bk_c = _bias_col(nc, spool, dr["bk"][l], "b_k")
            bvr = _bias_row(nc, rpool, dr["bv"][l])
            wq = _load_w_full(nc, wpool, dr["Wq"][l])
            wk = _load_w_full(nc, wpool, dr["Wk"][l])
            wv = _load_w_full(nc, wpool, dr["Wv"][l])
            for hb in range(BL):
                o = hb * S
                for w, bc, dst in ((wq, bq_c, qT), (wk, bk_c, kT)):
                    for m in range(KT):
                        ps = ppool.tile([P, S], F32, tag="pp", bufs=2,
                                        space="PSUM")
                        for kt in range(KT):
                            nc.tensor.matmul(ps[:], w[:, kt, m * P : (m + 1) * P],
                                             x[:, kt, o : o + S],
                                             start=(kt == 0), stop=(kt == KT - 1))
                        nc.scalar.activation(dst[:, m, o : o + S], ps[:],
                                             AF.Identity, bias=bc[:, m : m + 1])
                # V (token-major) for this half's two chunks
                for c in (2 * hb, 2 * hb + 1):
                    for hh in range(2):
                        ps = ppool.tile([P, 384], F32, tag="pp", bufs=2,
                                        space="PSUM")
                        for kt in range(KT):
                            nc.tensor.matmul(
                                ps[:], x[:, kt, c * P : (c + 1) * P],
                                wv[:, kt, hh * 384 : (hh + 1) * 384],
                                start=(kt == 0), stop=False)
                        nc.tensor.matmul(ps[:], ones_row[:1, :P],
                                         bvr[:1, hh * 384 : (hh + 1) * 384],
                                         start=False, stop=True)
                        nc.vector.tensor_copy(
                            out=vsb[:, c, hh * 6 : (hh + 1) * 6, :],
                            in_=ps[:].rearrange("p (h d) -> p h d", d=DH))
            wo = _load_w_full(nc, wpool, dr["Wo"][l])
            bo_c = _bias_col(nc, spool, dr["bo"][l], "bo_c")
            ag = _bias_col(nc, spool, dr["alg"][l], "ag")
            ab = _bias_col(nc, spool, dr["alb"][l], "ab")
            # attention per (half, head): raw context + per-head denominators
            # accumulated in a [12, 256] PSUM bank (row h = head h) via
            # one-hot lhsT columns; normalization happens once per half.
            for b in range(BL):
                o = b * S
                ps_D = ppool.tile([A, S], F32, tag="d_ps", bufs=2,
                                  space="PSUM")
                nc.tensor.matmul(ps_D[:], zero12[:], ones256r[:],
                                 start=True, stop=False, skip_group_check=True)
                for h in range(A):
                    ft, fo = h // 2, (h % 2) * DH
                    esb = []
                    for kc in range(2):
                        ps_s = ppool.tile([P, S], F32, tag="s_ps", bufs=3,
                                          space="PSUM")
                        nc.tensor.matmul(
                            ps_s[:],
                            kT[fo : fo + DH, ft, o + kc * P : o + (kc + 1) * P],
                            qT[fo : fo + DH, ft, o : o + S],
                            start=True, stop=True)
                        e = epool.tile([P, S], F32R, tag="e_sb", bufs=6,
                                       name="e_sb")
                        nc.scalar.activation(
                            e[:], ps_s[:], AF.Exp, scale=ISCALE,
                            bias=amask[:, b * 2 + kc : b * 2 + kc + 1])
                        esb.append(e)
                    ps_c = ppool.tile([DH, S], F32, tag="c_ps", bufs=1,
                                      space="PSUM")
                    for kc in range(2):
                        nc.tensor.matmul(ps_c[:], vsb[:, b * 2 + kc, h],
                                         esb[kc][:], start=(kc == 0),
                                         stop=(kc == 1))
                    nc.vector.tensor_copy(
                        out=craw[fo : fo + DH, ft, o : o + S], in_=ps_c[:])
                    for kc in range(2):
                        nc.tensor.matmul(ps_D[:], onehot12[:, h, :],
                                         esb[kc][:], start=False,
                                         stop=(h == A - 1 and kc == 1),
                                         skip_group_check=True)
                drec = spool.tile([A, S], F32R, tag="drec", bufs=2,
                                  name="drec")
                nc.vector.reciprocal(drec[:], ps_D[:])
                for ft in range(KT):
                    ps_bc = ppool.tile([P, S], F32, tag="pp", bufs=2,
                                       space="PSUM")
                    nc.tensor.matmul(ps_bc[:], dsel[:, ft], drec[:],
                                     start=True, stop=True)
                    nc.vector.tensor_tensor(
                        out=ctxT[:, ft, o : o + S], in0=ps_bc[:],
                        in1=_f(craw[:, ft, o : o + S]), op=ALU.mult)
                # Wo + residual (bias folded into DVE) + LN1, this half
                for m in range(KT):
                    ps = ppool.tile([P, S], F32, tag="pp", bufs=2,
                                    space="PSUM")
                    for kt in range(KT):
                        nc.tensor.matmul(ps[:], wo[:, kt, m * P : (m + 1) * P],
                                         ctxT[:, kt, o : o + S],
                                         start=(kt == 0), stop=(kt == KT - 1))
                    nc.vector.scalar_tensor_tensor(
                        out=y1[:, m, o : o + S], in0=ps[:],
                        scalar=bo_c[:, m : m + 1], in1=_f(x[:, m, o : o + S]),
                        op0=ALU.add, op1=ALU.add)
                emit_ln_half(nc, ppool, mpool, spool, y1, b, ag, ab, x2,
                             consts)

        # ---- phase 2: FFN (y2 in 6 persistent PSUM banks, halves
        # interleaved per chunk so each W1/W2 chunk is loaded once) + LN2 ----
        y3 = mpool.tile([P, KT, 512], F32R, tag="bigE", name="y3")
        xn = mpool.tile([P, KT, 512], F32R, tag="x_cur", name="xn")
        with (
            tc.tile_pool(name=f"ffa{l}", bufs=1, space="PSUM") as papool,
            tc.tile_pool(name=f"ffh{l}", bufs=1, space="PSUM") as fpool,
        ):
            b2_c = _bias_col(nc, spool, dr["b2"][l], "b2_c")
            # b1 in feature-major column layout [128, 24]: folded into Gelu
            b1_c = spool.tile([P, 4 * KT], F32, tag="b1_c", name="b1_c")
            nc.sync.dma_start(b1_c[:], dr["b1"][l].rearrange("(kt p) -> p kt", p=P))
            fg = _bias_col(nc, spool, dr["flg"][l], "fg")
            fb = _bias_col(nc, spool, dr["flb"][l], "fb")
            ps_y = [papool.tile([P, 512], F32, tag=f"acc{m}",
                                name=f"ps_y{l}_{m}", space="PSUM")
                    for m in range(KT)]
            first_mm = {}
            for q4 in range(4):
                w1 = _load_w_full(nc, wpool, dr["W1"][l][:, q4 * H : (q4 + 1) * H])
                for cc in range(KT):
                    c = q4 * KT + cc
                    w2 = w2pool.tile([P, H], F32R, tag="w2c", name="w2")
                    weng = nc.sync if c % 2 == 0 else nc.scalar
                    weng.dma_start(w2[:],
                                   _r(dr["W2"][l][c * P : (c + 1) * P, :]))
                    for hb in range(BL):
                        o = hb * S
                        ps_h = fpool.tile([P, S], F32, tag="h_ps", bufs=2,
                                          space="PSUM")
                        for kt in range(KT):
                            nc.tensor.matmul(ps_h[:],
                                             w1[:, kt, cc * P : (cc + 1) * P],
                                             x2[:, kt, o : o + S],
                                             start=(kt == 0), stop=(kt == KT - 1))
                        hsb = hpool.tile([P, S], F32R, tag="h_sb", bufs=4,
                                         name="hsb")
                        nc.scalar.activation(hsb[:], ps_h[:],
                                             AF.Gelu_apprx_tanh,
                                             bias=b1_c[:, c : c + 1])
                        for m in range(KT):
                            mm = nc.tensor.matmul(
                                ps_y[m][:, o : o + S],
                                w2[:, m * P : (m + 1) * P], hsb[:],
                                start=(c == 0 and hb == 0),
                                stop=(c == 4 * KT - 1 and hb == 1),
                                skip_group_check=True)
                            if c == 0:
                                if hb == 0:
                                    first_mm[m] = mm
                                else:
                                    add_dep_helper(mm.ins, first_mm[m].ins,
                                                   reason="acc after bank clear")
            for hb in range(BL):
                o = hb * S
                for m in range(KT):
                    nc.vector.scalar_tensor_tensor(
                        out=y3[:, m, o : o + S], in0=ps_y[m][:, o : o + S],
                        scalar=b2_c[:, m : m + 1], in1=_f(x2[:, m, o : o + S]),
                        op0=ALU.add, op1=ALU.add)
                emit_ln_half(nc, fpool, mpool, spool, y3, hb, fg, fb, xn,
                             consts, ltag="h_ps")
        x = xn

    # ============ classifier head + softmax + compaction ============
    with tc.tile_pool(name="head", bufs=2, space="PSUM") as ppool:
        clsw = spool.tile([P, KT, NL], F32, tag="clsw", name="clsw")
        nc.sync.dma_start(clsw[:], dr["clsW"].rearrange("(kt p) c -> p kt c", p=P))
        clsb = _bias_row(nc, rpool, dr["clsb"][:], tag="clsb", dtype=F32)

        # uniform pad row: softmax(cls_b), broadcast to 128 partitions
        nmx = spool.tile([1, 1], F32, tag="nmx", name="nmx")
        nc.vector.reduce_max(out=nmx[:], in_=clsb[:], negate=True,
                             axis=mybir.AxisListType.X)
        usum = spool.tile([1, 1], F32, tag="usum", name="usum")
        uex = spool.tile([1, NL], F32, tag="uex", name="uex")
        nc.scalar.activation(uex[:], clsb[:], AF.Exp, bias=nmx[:],
                             accum_out=usum[:])
        urs = spool.tile([1, 1], F32, tag="urs", name="urs")
        nc.vector.reciprocal(urs[:], usum[:])
        uni = spool.tile([1, NL], F32, tag="uni", name="uni")
        nc.vector.tensor_scalar_mul(uni[:], uex[:], urs[:])
        ps_u = ppool.tile([P, NL], F32, tag="u_ps", space="PSUM")
        nc.tensor.matmul(ps_u[:], ones_f32[:1, :P], uni[:], start=True, stop=True)
        uni128 = spool.tile([P, NL], F32, tag="uni128", name="uni128")
        nc.vector.tensor_copy(out=uni128[:], in_=ps_u[:])
        out_flat = dr["out"].rearrange("b s c -> (b s) c")
        prefills = []
        for c in range(TC):
            dma = nc.sync.dma_start(out_flat[c * P : (c + 1) * P, :], uni128[:])
            prefills.append(dma.ins)

        for c in range(TC):
            b = c // (S // P)
            ps_lg = ppool.tile([P, NL], F32, tag="lg_ps", space="PSUM")
            for kt in range(KT):
                nc.tensor.matmul(ps_lg[:], _f(x[:, kt, c * P : (c + 1) * P]),
                                 clsw[:, kt], start=(kt == 0), stop=False)
            nc.tensor.matmul(ps_lg[:], ones_f32[:1, :P], clsb[:],
                             start=False, stop=True)
            negmax = spool.tile([P, 1], F32, tag="negmax", name="negmax")
            nc.vector.reduce_max(out=negmax[:], in_=ps_lg[:], negate=True,
                                 axis=mybir.AxisListType.X)
            probs = spool.tile([P, NL], F32, tag="probs", name="probs")
            sm = spool.tile([P, 1], F32, tag="sm", name="sm")
            nc.scalar.activation(probs[:], ps_lg[:], AF.Exp, bias=negmax[:],
                                 accum_out=sm[:])
            rs = spool.tile([P, 1], F32, tag="rs", name="rs")
            nc.vector.reciprocal(rs[:], sm[:])
            nc.vector.tensor_scalar_mul(probs[:], probs[:], rs[:])

            # cumsum of valid over the sequence, sliced to this chunk
            cc = c % (S // P)
            ps_cs = ppool.tile([P, 1], F32, tag="cs_ps", space="PSUM")
            lhs_ks = [ones_f32[:, :P] if ks < cc else ltri
                      for ks in range(cc + 1)]
            for ks, lhs in enumerate(lhs_ks):
                nc.tensor.matmul(ps_cs[:], lhs[:],
                                 valid_f[:, b * 2 + ks : b * 2 + ks + 1],
                                 start=(ks == 0), stop=(ks == len(lhs_ks) - 1))
            # dest = valid ? b*S + csum - 1 : BIG
            dest_f = spool.tile([P, 1], F32, tag="dest_f", name="dest_f")
            nc.vector.tensor_scalar_add(dest_f[:], ps_cs[:], float(b * S - 1 - BIG))
            nc.vector.tensor_tensor(out=dest_f[:], in0=dest_f[:],
                                    in1=valid_f[:, c : c + 1], op=ALU.mult)
            nc.vector.tensor_scalar_add(dest_f[:], dest_f[:], float(BIG))
            dest_i = spool.tile([P, 1], I32, tag="dest_i", name="dest_i")
            nc.vector.tensor_copy(out=dest_i[:], in_=dest_f[:])

            scat = nc.gpsimd.indirect_dma_start(
                out=out_flat[:, :],
                out_offset=bass.IndirectOffsetOnAxis(ap=dest_i[:, :1], axis=0),
                in_=probs[:],
                in_offset=None,
                bounds_check=T - 1, oob_is_err=False,
            )
            for pf in prefills:
                add_dep_helper(scat.ins, pf,
                               reason="scatter after uniform prefill")


_NC_CACHE = {}


def _get_nc(repeat=1, n_layers=L):
    key = (repeat, n_layers)
    if key not in _NC_CACHE:
        _NC_CACHE[key] = build_nc(repeat=repeat, n_layers=n_layers)
    return _NC_CACHE[key]


def make_in_maps(inputs):
    per_seq = {}
    for name in ("input_word_ids", "input_mask", "input_type_ids", "valid_mask"):
        per_seq[name] = np.ascontiguousarray(np.asarray(inputs[name]))
    shared = {
        k: np.ascontiguousarray(np.asarray(v))
        for k, v in inputs.items()
        if k not in per_seq
    }
    in_maps = []
    for c in range(NC):
        m = dict(shared)
        for name, arr in per_seq.items():
            m[name] = np.ascontiguousarray(arr[c * BL : (c + 1) * BL])
        in_maps.append(m)
    return in_maps


def kernel(**inputs):
    nc = _get_nc()
    in_maps = make_in_maps(inputs)
    res = bass_utils.run_bass_kernel_spmd(nc, in_maps, list(range(NC)))
    out = np.concatenate([res.results[c]["out"] for c in range(NC)], axis=0)
    return out.astype(np.float32)



# revision 34
# speedup vs baseline: 1.0694x; 1.0694x over previous
"""BERT-NER (12-layer BERT-base + token compaction + classifier) on 8 TRN2 cores.

Data-parallel over batch: 16 sequences -> 2 per core. Weights replicated.
Activations are kept feature-major (xT: [768 partitions(6 tiles), 512 tokens])
so every x@W matmul uses the stored W[in,out] directly as lhsT.
All matmuls run in float32r (full PE rate for N>=256), storage stays fp32.
LayerNorm stats, softmax row-sums and free-dim biases are folded into small
matmuls (ones vectors) to avoid partition-dim reductions on DVE.
"""

import os
import sys

for _p in ("/opt/trn_rl_repo", "/root/.axon_site/_ro/trn_rl_repo"):
    if os.path.isdir(_p) and _p not in sys.path:
        sys.path.insert(0, _p)

import numpy as np

import concourse.bass as bass
import concourse.mybir as mybir
import concourse.tile as tile
from concourse.tile import add_dep_helper
from concourse import bacc, bass_utils

F32 = mybir.dt.float32
F32R = mybir.dt.float32r
I32 = mybir.dt.int32
AF = mybir.ActivationFunctionType
ALU = mybir.AluOpType

B, S, H, L, A, V, NL = 16, 256, 768, 12, 12, 30522, 9
DH = H // A  # 64
FF = 4 * H  # 3072
NC = 8  # cores
BL = B // NC  # 2 sequences per core
T = BL * S  # 512 tokens per core
KT = H // 128  # 6 k-tiles of the hidden dim
TC = T // 128  # 4 token chunks
BIG = 1_000_000  # OOB dump index for compaction scatter
EPS = 1e-12
ISCALE = 1.0 / float(np.sqrt(DH))

P = 128


def _r(ap):
    """View an AP as float32r (bit-identical, PE reduced precision)."""
    return ap.bitcast(F32R)


def _f(ap):
    """View an f32r AP as plain float32 for DVE/ACT reads."""
    return ap.bitcast(F32)


def build_nc(repeat=1, n_layers=L):
    nc = bacc.Bacc("TRN2", target_bir_lowering=False, debug=False)

    d_ids = nc.dram_tensor("input_word_ids", [BL, S], I32, kind="ExternalInput")
    d_mask = nc.dram_tensor("input_mask", [BL, S], I32, kind="ExternalInput")
    d_type = nc.dram_tensor("input_type_ids", [BL, S], I32, kind="ExternalInput")
    d_valid = nc.dram_tensor("valid_mask", [BL, S], I32, kind="ExternalInput")
    d_wemb = nc.dram_tensor("word_emb", [V, H], F32, kind="ExternalInput")
    d_pemb = nc.dram_tensor("pos_emb", [S, H], F32, kind="ExternalInput")
    d_temb = nc.dram_tensor("type_emb", [2, H], F32, kind="ExternalInput")
    d_elng = nc.dram_tensor("emb_ln_g", [H], F32, kind="ExternalInput")
    d_elnb = nc.dram_tensor("emb_ln_b", [H], F32, kind="ExternalInput")
    d_Wq = nc.dram_tensor("Wq", [L, H, H], F32, kind="ExternalInput")
    d_bq = nc.dram_tensor("bq", [L, H], F32, kind="ExternalInput")
    d_Wk = nc.dram_tensor("Wk", [L, H, H], F32, kind="ExternalInput")
    d_bk = nc.dram_tensor("bk", [L, H], F32, kind="ExternalInput")
    d_Wv = nc.dram_tensor("Wv", [L, H, H], F32, kind="ExternalInput")
    d_bv = nc.dram_tensor("bv", [L, H], F32, kind="ExternalInput")
    d_Wo = nc.dram_tensor("Wo", [L, H, H], F32, kind="ExternalInput")
    d_bo = nc.dram_tensor("bo", [L, H], F32, kind="ExternalInput")
    d_alg = nc.dram_tensor("attn_ln_g", [L, H], F32, kind="ExternalInput")
    d_alb = nc.dram_tensor("attn_ln_b", [L, H], F32, kind="ExternalInput")
    d_W1 = nc.dram_tensor("W1", [L, H, FF], F32, kind="ExternalInput")
    d_b1 = nc.dram_tensor("b1", [L, FF], F32, kind="ExternalInput")
    d_W2 = nc.dram_tensor("W2", [L, FF, H], F32, kind="ExternalInput")
    d_b2 = nc.dram_tensor("b2", [L, H], F32, kind="ExternalInput")
    d_flg = nc.dram_tensor("ffn_ln_g", [L, H], F32, kind="ExternalInput")
    d_flb = nc.dram_tensor("ffn_ln_b", [L, H], F32, kind="ExternalInput")
    d_clsW = nc.dram_tensor("cls_W", [H, NL], F32, kind="ExternalInput")
    d_clsb = nc.dram_tensor("cls_b", [NL], F32, kind="ExternalInput")
    d_out = nc.dram_tensor("out", [BL, S, NL], F32, kind="ExternalOutput")

    dr = dict(
        ids=d_ids, mask=d_mask, type=d_type, valid=d_valid, wemb=d_wemb,
        pemb=d_pemb, temb=d_temb, elng=d_elng, elnb=d_elnb,
        Wq=d_Wq, bq=d_bq, Wk=d_Wk, bk=d_bk, Wv=d_Wv, bv=d_bv, Wo=d_Wo, bo=d_bo,
        alg=d_alg, alb=d_alb, W1=d_W1, b1=d_b1, W2=d_W2, b2=d_b2,
        flg=d_flg, flb=d_flb, clsW=d_clsW, clsb=d_clsb, out=d_out,
    )

    with nc.allow_low_precision(reason="float32r matmul pipeline"), tile.TileContext(
        nc
    ) as tc:
        with (
            tc.tile_pool(name="const", bufs=1) as cpool,
            tc.tile_pool(name="main", bufs=1) as mpool,
            tc.tile_pool(name="wts", bufs=5) as wpool,
            tc.tile_pool(name="w2p", bufs=6) as w2pool,
            tc.tile_pool(name="hrows", bufs=2) as rpool,
            tc.tile_pool(name="hbuf", bufs=3) as hpool,
            tc.tile_pool(name="ebuf", bufs=4) as epool,
            tc.tile_pool(name="small", bufs=2) as spool,
        ):
            pools = dict(c=cpool, m=mpool, w=wpool, w2=w2pool, r=rpool,
                         h=hpool, e=epool, s=spool)
            # ---- constants (device-generated) ----
            ident = cpool.tile([P, P], F32, tag="ident")
            nc.gpsimd.memset(ident[:], 0.0)
            nc.gpsimd.affine_select(
                out=ident[:], in_=ident[:], compare_op=ALU.not_equal, fill=1.0,
                base=0, pattern=[[-1, P]], channel_multiplier=1,
            )
            ones_f32 = cpool.tile([P, 512], F32, tag="ones_f32")
            nc.gpsimd.memset(ones_f32[:], 1.0)
            ones_row = cpool.tile([1, 512], F32R, tag="ones_row")
            nc.vector.tensor_copy(out=ones_row[:], in_=ones_f32[:1, :])
            ones128 = cpool.tile([P, P], F32R, tag="ones128")
            nc.vector.tensor_copy(out=ones128[:], in_=ones_f32[:, :P])
            ones256r = cpool.tile([P, S], F32R, tag="ones256r")
            nc.vector.tensor_copy(out=ones256r[:], in_=ones_f32[:, :S])
            # onehot12[:, h, :]: [128, 12] lhsT whose column h is all-ones --
            # rowsum matmul lands head h's softmax denominator on partition h.
            # (built in f32, then DVE-copied to f32r: fp32r matmul operands
            # must come from a rounding producer, the BIR verifier checks.)
            onehot12f = cpool.tile([P, A, A], F32, tag="cscratch")
            nc.gpsimd.memset(onehot12f[:], 1.0)
            nc.gpsimd.affine_select(
                out=onehot12f[:], in_=onehot12f[:], compare_op=ALU.is_equal,
                fill=0.0, base=0, pattern=[[-1, A], [1, A]],
                channel_multiplier=0,
            )
            onehot12 = cpool.tile([P, A, A], F32R, tag="onehot12")
            nc.vector.tensor_copy(out=onehot12[:], in_=onehot12f[:])
            zero12f = cpool.tile([P, A], F32, tag="cscratch")
            nc.gpsimd.memset(zero12f[:], 0.0)
            zero12 = cpool.tile([P, A], F32R, tag="zero12")
            nc.vector.tensor_copy(out=zero12[:], in_=zero12f[:])
            # dsel[:, ft]: [12, 128] lhsT; column m is one-hot at row
            # 2*ft + (m >= 64) -- broadcasts the two per-head denominator rows
            # of tile ft across its 2x64 partitions.
            dself = cpool.tile([A, KT, 2, DH], F32, tag="cscratch")
            nc.gpsimd.memset(dself[:], 1.0)
            nc.gpsimd.affine_select(
                out=dself[:], in_=dself[:], compare_op=ALU.is_equal, fill=0.0,
                base=0, pattern=[[-2, KT], [-1, 2], [0, DH]],
                channel_multiplier=1,
            )
            dsel = cpool.tile([A, KT, 2, DH], F32R, tag="dsel")
            nc.vector.tensor_copy(out=dsel[:], in_=dself[:])
            # lower-triangular-inclusive block: tri[p, t] = 1 if p <= t
            ltri_f = cpool.tile([P, P], F32, tag="ltri_f")
            nc.gpsimd.memset(ltri_f[:], 1.0)
            nc.gpsimd.affine_select(
                out=ltri_f[:], in_=ltri_f[:], compare_op=ALU.is_ge, fill=0.0,
                base=0, pattern=[[1, P]], channel_multiplier=-1,
            )
            c_eps = cpool.tile([P, 1], F32, tag="c_eps")
            nc.gpsimd.memset(c_eps[:], EPS)
            consts = dict(ident=ident, ones_row=ones_row, ltri=ltri_f,
                          c_eps=c_eps, ones_f32=ones_f32, ones128=ones128,
                          ones256r=ones256r, onehot12=onehot12,
                          zero12=zero12, dsel=dsel)

            def body():
                emit_body(nc, tc, pools, consts, dr, n_layers)

            if repeat == 1:
                body()
            else:
                with tc.For_i(0, repeat, 1):
                    body()

    nc.compile()
    return nc


BF16 = mybir.dt.bfloat16


def _load_w_full(nc, wpool, d_slice):
    """Load a [H, 768] DRAM slice as SBUF [128, KT, 768] bf16 (k-tiles on
    partitions). fp32->bf16 cast happens inline in the SWDGE DMA; bf16
    halves the SBUF footprint (deeper prefetch) and enables FWL fast
    weight loads on the PE."""
    w = wpool.tile([P, KT, H], BF16, tag="w_big", name="w_big")
    src = d_slice.rearrange("(kt p) c -> p kt c", p=P)
    nc.gpsimd.dma_start(w[:, 0:3], src[:, 0:3])
    nc.gpsimd.dma_start(w[:, 3:6], src[:, 3:6])
    return w


def _bias_col(nc, spool, d_vec, tag):
    """Load [H] DRAM vector as [128, KT] (col m = slice m*128:(m+1)*128)."""
    t = spool.tile([P, KT], F32, tag=tag, name=tag)
    nc.sync.dma_start(t[:], d_vec.rearrange("(kt p) -> p kt", p=P))
    return t


def _bias_row(nc, rpool, d_vec, tag="brow", dtype=F32R):
    """Load a DRAM vector [N<=768] as a single-partition row [1, N]."""
    n = d_vec.shape[0]
    t = rpool.tile([1, n], dtype, tag=tag, name=tag)
    nc.sync.dma_start(t[:], d_vec[None, :].bitcast(dtype))
    return t


def emit_ln_half(nc, ppool, mpool, spool, y, hb, g_col, b_col, out, consts,
                 ltag="pp"):
    """Feature-major layernorm over token half hb (columns hb*S..hb*S+S).
    Stat matmuls use an all-ones [128,128] lhsT so per-token sums land
    broadcast across 128 partitions; sum and sum-of-squares share one PSUM
    bank (two regions), ordered by an explicit dep after the bank clear."""
    ones128 = consts["ones128"]
    o = hb * S
    ln = ppool.tile([P, 2, S], F32, tag=ltag, bufs=2, space="PSUM",
                    name="ln_ps")
    mm_clear = None
    for kt in range(KT):
        sq = mpool.tile([P, S], F32R, tag="ln_sq", bufs=2, name="sq")
        nc.scalar.activation(sq[:], _f(y[:, kt, o : o + S]), AF.Square)
        mm1 = nc.tensor.matmul(ln[:, 0], ones128[:], y[:, kt, o : o + S],
                               start=(kt == 0), stop=(kt == KT - 1),
                               skip_group_check=True)
        mm2 = nc.tensor.matmul(ln[:, 1], ones128[:], sq[:],
                               start=False, stop=(kt == KT - 1),
                               skip_group_check=True)
        if kt == 0:
            add_dep_helper(mm2.ins, mm1.ins, reason="sumsq after bank clear")
    mean = spool.tile([P, S], F32, tag="ln_mean", bufs=2, name="ln_mean")
    nc.vector.tensor_scalar_mul(mean[:], ln[:, 0], 1.0 / H)
    m2 = spool.tile([P, S], F32, tag="ln_m2", bufs=1, name="ln_m2")
    nc.vector.tensor_tensor(out=m2[:], in0=mean[:], in1=mean[:], op=ALU.mult)
    var = spool.tile([P, S], F32, tag="ln_var", bufs=1, name="ln_var")
    nc.vector.scalar_tensor_tensor(out=var[:], in0=ln[:, 1], scalar=1.0 / H,
                                   in1=m2[:], op0=ALU.mult, op1=ALU.subtract)
    std = spool.tile([P, S], F32, tag="ln_std", bufs=1, name="ln_std")
    sqrt_ins = nc.scalar.activation(std[:], var[:], AF.Sqrt,
                                    bias=consts["c_eps"][:])
    rstd = spool.tile([P, S], F32, tag="ln_rstd", bufs=2, name="ln_rstd")
    nc.vector.reciprocal(rstd[:], std[:])
    for kt in range(KT):
        tmp = mpool.tile([P, S], F32, tag="ln_tmp", bufs=2, name="tmp")
        nc.vector.tensor_tensor(out=tmp[:], in0=_f(y[:, kt, o : o + S]),
                                in1=mean[:], op=ALU.subtract)
        nc.vector.tensor_tensor(out=tmp[:], in0=tmp[:], in1=rstd[:],
                                op=ALU.mult)
        nc.scalar.activation(out[:, kt, o : o + S], tmp[:], AF.Identity,
                             scale=g_col[:, kt : kt + 1],
                             bias=b_col[:, kt : kt + 1])
    return sqrt_ins


def emit_body(nc, tc, pools, consts, dr, n_layers):
    cpool, mpool, wpool, w2pool = (
        pools["c"], pools["m"], pools["w"], pools["w2"])
    rpool, hpool, epool, spool = (
        pools["r"], pools["h"], pools["e"], pools["s"])
    ident, ones_row, ltri = (
        consts["ident"], consts["ones_row"], consts["ltri"])
    ones_f32 = consts["ones_f32"]
    onehot12, zero12, dsel = (
        consts["onehot12"], consts["zero12"], consts["dsel"])
    ones256r = consts["ones256r"]

    ids_flat = dr["ids"].rearrange("b s -> (b s)")
    type_flat = dr["type"].rearrange("b s -> (b s)")
    mask_flat = dr["mask"].rearrange("b s -> (b s)")
    valid_flat = dr["valid"].rearrange("b s -> (b s)")

    # amask[:, c]: 0 where mask==1 else -10000 ; valid_f: valid mask as f32r
    amask = cpool.tile([P, TC], F32, tag="amask", name="amask")
    valid_f = cpool.tile([P, TC], F32, tag="valid_f", name="valid_f")

    # ============ embeddings (token-major), transpose, LN ============
    xtok = mpool.tile([P, TC, H], F32, tag="bigA", name="xtok")
    for c in range(TC):
        idt = spool.tile([P, 1], I32, tag="idt", name="idt")
        nc.sync.dma_start(idt[:], ids_flat[c * P : (c + 1) * P, None])
        nc.gpsimd.indirect_dma_start(
            out=xtok[:, c], out_offset=None, in_=dr["wemb"][:, :],
            in_offset=bass.IndirectOffsetOnAxis(ap=idt[:, :1], axis=0),
        )
        tyt = spool.tile([P, 1], I32, tag="tyt", name="tyt")
        nc.sync.dma_start(tyt[:], type_flat[c * P : (c + 1) * P, None])
        # accumulate type + position embeddings in-flight (SWDGE CCE add)
        nc.gpsimd.indirect_dma_start(
            out=xtok[:, c], out_offset=None, in_=dr["temb"][:, :],
            in_offset=bass.IndirectOffsetOnAxis(ap=tyt[:, :1], axis=0),
            compute_op=ALU.add,
        )
        cc = c % (S // P)
        nc.gpsimd.dma_start(xtok[:, c], dr["pemb"][cc * P : (cc + 1) * P, :],
                            accum_op=ALU.add)

        mi = spool.tile([P, 1], I32, tag="mi", name="mi")
        nc.sync.dma_start(mi[:], mask_flat[c * P : (c + 1) * P, None])
        mf = spool.tile([P, 1], F32, tag="mf", name="mf")
        nc.vector.tensor_copy(out=mf[:], in_=mi[:])
        nc.scalar.activation(amask[:, c : c + 1], mf[:], AF.Copy,
                             scale=10000.0, bias=-10000.0)
        vi = spool.tile([P, 1], I32, tag="vi", name="vi")
        nc.sync.dma_start(vi[:], valid_flat[c * P : (c + 1) * P, None])
        nc.vector.tensor_copy(out=valid_f[:, c : c + 1], in_=vi[:])

    # transpose to feature-major
    xe = mpool.tile([P, KT, 512], F32R, tag="bigB", name="xe")
    x = mpool.tile([P, KT, 512], F32R, tag="x_cur", name="x0")
    with tc.tile_pool(name="embtr", bufs=1, space="PSUM") as ppool:
        for kt in range(KT):
            for c in range(TC):
                ps_t = ppool.tile([P, P], F32, tag="tr", bufs=2, space="PSUM")
                nc.tensor.transpose(
                    out=ps_t[:], in_=xtok[:, c, kt * P : (kt + 1) * P],
                    identity=ident[:])
                nc.vector.tensor_copy(out=xe[:, kt, c * P : (c + 1) * P],
                                      in_=ps_t[:])
        eg = _bias_col(nc, spool, dr["elng"][:], "eg")
        eb = _bias_col(nc, spool, dr["elnb"][:], "eb")
        for hb in range(BL):
            emit_ln_half(nc, ppool, mpool, spool, xe, hb, eg, eb, x, consts)

    # ============ transformer layers ============
    for l in range(n_layers):
        # ---- Q, K projections (feature-major out) ----
        qT = mpool.tile([P, KT, 512], F32R, tag="bigA", name="qT")
        kT = mpool.tile([P, KT, 512], F32R, tag="bigB", name="kT")
        vsb = mpool.tile([P, TC, A, DH], F32R, tag="vsb", name="vsb")
        craw = mpool.tile([P, KT, 512], F32R, tag="bigC", name="craw")
        ctxT = mpool.tile([P, KT, 512], F32R, tag="bigD", name="ctxT")
        y1 = mpool.tile([P, KT, 512], F32R, tag="bigE", name="y1")
        x2 = mpool.tile([P, KT, 512], F32R, tag="bigF", name="x2")

        # ---- phase 1: QKV projections + attention + Wo + LN1, pipelined by
        # sequence half (the two sequences are independent until the FFN's
        # shared weight streaming). All matmuls are N=256 (full f32r rate).
        with tc.tile_pool(name=f"ph1_{l}", bufs=1, space="PSUM") as ppool:
            bq_c = _bias_col(nc, spool, dr["bq"][l], "b_q")
            bk_c = _bias_col(nc, spool, dr["bk"][l], "b_k")
            bvr = _bias_row(nc, rpool, dr["bv"][l])
            wq = _load_w_full(nc, wpool, dr["Wq"][l])
            wk = _load_w_full(nc, wpool, dr["Wk"][l])
            wv = _load_w_full(nc, wpool, dr["Wv"][l])
            for hb in range(BL):
                o = hb * S
                for w, bc, dst in ((wq, bq_c, qT), (wk, bk_c, kT)):
                    for m in range(KT):
                        ps = ppool.tile([P, S], F32, tag="pp", bufs=2,
                                        space="PSUM")
                        for kt in range(KT):
                            nc.tensor.matmul(ps[:], w[:, kt, m * P : (m + 1) * P],
                                             x[:, kt, o : o + S],
                                             start=(kt == 0), stop=(kt == KT - 1))
                        nc.scalar.activation(dst[:, m, o : o + S], ps[:],
                                             AF.Identity, bias=bc[:, m : m + 1])
                # V (token-major) for this half's two chunks
                for c in (2 * hb, 2 * hb + 1):
                    for hh in range(2):
                        ps = ppool.tile([P, 384], F32, tag="pp", bufs=2,
                                        space="PSUM")
                        for kt in range(KT):
                            nc.tensor.matmul(
                                ps[:], x[:, kt, c * P : (c + 1) * P],
                                wv[:, kt, hh * 384 : (hh + 1) * 384],
                                start=(kt == 0), stop=False)
                        nc.tensor.matmul(ps[:], ones_row[:1, :P],
                                         bvr[:1, hh * 384 : (hh + 1) * 384],
                                         start=False, stop=True)
                        nc.vector.tensor_copy(
                            out=vsb[:, c, hh * 6 : (hh + 1) * 6, :],
                            in_=ps[:].rearrange("p (h d) -> p h d", d=DH))
            wo = _load_w_full(nc, wpool, dr["Wo"][l])
            bo_c = _bias_col(nc, spool, dr["bo"][l], "bo_c")
            ag = _bias_col(nc, spool, dr["alg"][l], "ag")
            ab = _bias_col(nc, spool, dr["alb"][l], "ab")
            # attention per (half, head): raw context + per-head denominators
            # accumulated in a [12, 256] PSUM bank (row h = head h) via
            # one-hot lhsT columns; normalization happens once per half.
            ln1_sqrt0 = None
            last_exp = [None]
            for b in range(BL):
                o = b * S
                ps_D = ppool.tile([A, S], F32, tag="d_ps", bufs=2,
                                  space="PSUM")
                nc.tensor.matmul(ps_D[:], zero12[:], ones256r[:],
                                 start=True, stop=False, skip_group_check=True)
                for h in range(A):
                    ft, fo = h // 2, (h % 2) * DH
                    esb = []
                    for kc in range(2):
                        ps_s = ppool.tile([P, S], F32, tag="s_ps", bufs=3,
                                          space="PSUM")
                        nc.tensor.matmul(
                            ps_s[:],
                            kT[fo : fo + DH, ft, o + kc * P : o + (kc + 1) * P],
                            qT[fo : fo + DH, ft, o : o + S],
                            start=True, stop=True)
                        e = epool.tile([P, S], F32R, tag="e_sb", bufs=6,
                                       name="e_sb")
                        last_exp[0] = nc.scalar.activation(
                            e[:], ps_s[:], AF.Exp, scale=ISCALE,
                            bias=amask[:, b * 2 + kc : b * 2 + kc + 1])
                        esb.append(e)
                    ps_c = ppool.tile([DH, S], F32, tag="c_ps", bufs=1,
                                      space="PSUM")
                    for kc in range(2):
                        nc.tensor.matmul(ps_c[:], vsb[:, b * 2 + kc, h],
                                         esb[kc][:], start=(kc == 0),
                                         stop=(kc == 1))
                    nc.vector.tensor_copy(
                        out=craw[fo : fo + DH, ft, o : o + S], in_=ps_c[:])
                    for kc in range(2):
                        nc.tensor.matmul(ps_D[:], onehot12[:, h, :],
                                         esb[kc][:], start=False,
                                         stop=(h == A - 1 and kc == 1),
                                         skip_group_check=True)
                drec = spool.tile([A, S], F32R, tag="drec", bufs=2,
                                  name="drec")
                nc.vector.reciprocal(drec[:], ps_D[:])
                for ft in range(KT):
                    ps_bc = ppool.tile([P, S], F32, tag="pp", bufs=2,
                                       space="PSUM")
                    nc.tensor.matmul(ps_bc[:], dsel[:, ft], drec[:],
                                     start=True, stop=True)
                    nc.vector.tensor_tensor(
                        out=ctxT[:, ft, o : o + S], in0=ps_bc[:],
                        in1=_f(craw[:, ft, o : o + S]), op=ALU.mult)
                # Wo + residual (bias folded into DVE) + LN1, this half
                for m in range(KT):
                    ps = ppool.tile([P, S], F32, tag="pp", bufs=2,
                                    space="PSUM")
                    for kt in range(KT):
                        nc.tensor.matmul(ps[:], wo[:, kt, m * P : (m + 1) * P],
                                         ctxT[:, kt, o : o + S],
                                         start=(kt == 0), stop=(kt == KT - 1))
                    nc.vector.scalar_tensor_tensor(
                        out=y1[:, m, o : o + S], in0=ps[:],
                        scalar=bo_c[:, m : m + 1], in1=_f(x[:, m, o : o + S]),
                        op0=ALU.add, op1=ALU.add)
                s_ins = emit_ln_half(nc, ppool, mpool, spool, y1, b, ag, ab,
                                     x2, consts)
                if b == 0:
                    ln1_sqrt0 = s_ins
            # keep LN1(b0)'s Sqrt (foreign ACT table set) after the second
            # half's exps so the exp->sqrt->exp table reload ping-pong
            # collapses to one switch per layer
            if ln1_sqrt0 is not None and last_exp[0] is not None:
                add_dep_helper(ln1_sqrt0.ins, last_exp[0].ins,
                               reason="defer sqrt past exps (act table)")

        # ---- phase 2: FFN (y2 in 6 persistent PSUM banks, halves
        # interleaved per chunk so each W1/W2 chunk is loaded once) + LN2 ----
        y3 = mpool.tile([P, KT, 512], F32R, tag="bigE", name="y3")
        xn = mpool.tile([P, KT, 512], F32R, tag="x_cur", name="xn")
        with (
            tc.tile_pool(name=f"ffa{l}", bufs=1, space="PSUM") as papool,
            tc.tile_pool(name=f"ffh{l}", bufs=1, space="PSUM") as fpool,
        ):
            b2_c = _bias_col(nc, spool, dr["b2"][l], "b2_c")
            # b1 in feature-major column layout [128, 24]: folded into Gelu
            b1_c = spool.tile([P, 4 * KT], F32, tag="b1_c", name="b1_c")
            nc.sync.dma_start(b1_c[:], dr["b1"][l].rearrange("(kt p) -> p kt", p=P))
            fg = _bias_col(nc, spool, dr["flg"][l], "fg")
            fb = _bias_col(nc, spool, dr["flb"][l], "fb")
            ps_y = [papool.tile([P, 512], F32, tag=f"acc{m}",
                                name=f"ps_y{l}_{m}", space="PSUM")
                    for m in range(KT)]
            first_mm = {}
            for q4 in range(4):
                w1 = _load_w_full(nc, wpool, dr["W1"][l][:, q4 * H : (q4 + 1) * H])
                for cc in range(KT):
                    c = q4 * KT + cc
                    w2 = w2pool.tile([P, H], BF16, tag="w2c", name="w2")
                    nc.gpsimd.dma_start(w2[:],
                                        dr["W2"][l][c * P : (c + 1) * P, :])
                    for hb in range(BL):
                        o = hb * S
                        ps_h = fpool.tile([P, S], F32, tag="h_ps", bufs=2,
                                          space="PSUM")
                        for kt in range(KT):
                            nc.tensor.matmul(ps_h[:],
                                             w1[:, kt, cc * P : (cc + 1) * P],
                                             x2[:, kt, o : o + S],
                                             start=(kt == 0), stop=(kt == KT - 1))
                        hsb = hpool.tile([P, S], F32R, tag="h_sb", bufs=4,
                                         name="hsb")
                        nc.scalar.activation(hsb[:], ps_h[:],
                                             AF.Gelu_apprx_tanh,
                                             bias=b1_c[:, c : c + 1])
                        for m in range(KT):
                            mm = nc.tensor.matmul(
                                ps_y[m][:, o : o + S],
                                w2[:, m * P : (m + 1) * P], hsb[:],
                                start=(c == 0 and hb == 0),
                                stop=(c == 4 * KT - 1 and hb == 1),
                                skip_group_check=True)
                            if c == 0:
                                if hb == 0:
                                    first_mm[m] = mm
                                else:
                                    add_dep_helper(mm.ins, first_mm[m].ins,
                                                   reason="acc after bank clear")
            for hb in range(BL):
                o = hb * S
                for m in range(KT):
                    nc.vector.scalar_tensor_tensor(
                        out=y3[:, m, o : o + S], in0=ps_y[m][:, o : o + S],
                        scalar=b2_c[:, m : m + 1], in1=_f(x2[:, m, o : o + S]),
                        op0=ALU.add, op1=ALU.add)
                emit_ln_half(nc, fpool, mpool, spool, y3, hb, fg, fb, xn,
                             consts, ltag="h_ps")
        x = xn

    # ============ classifier head + softmax + compaction ============
    with tc.tile_pool(name="head", bufs=2, space="PSUM") as ppool:
        clsw = spool.tile([P, KT, NL], F32, tag="clsw", name="clsw")
        nc.sync.dma_start(clsw[:], dr["clsW"].rearrange("(kt p) c -> p kt c", p=P))
        clsb = _bias_row(nc, rpool, dr["clsb"][:], tag="clsb", dtype=F32)

        # uniform pad row: softmax(cls_b), broadcast to 128 partitions
        nmx = spool.tile([1, 1], F32, tag="nmx", name="nmx")
        nc.vector.reduce_max(out=nmx[:], in_=clsb[:], negate=True,
                             axis=mybir.AxisListType.X)
        usum = spool.tile([1, 1], F32, tag="usum", name="usum")
        uex = spool.tile([1, NL], F32, tag="uex", name="uex")
        nc.scalar.activation(uex[:], clsb[:], AF.Exp, bias=nmx[:],
                             accum_out=usum[:])
        urs = spool.tile([1, 1], F32, tag="urs", name="urs")
        nc.vector.reciprocal(urs[:], usum[:])
        uni = spool.tile([1, NL], F32, tag="uni", name="uni")
        nc.vector.tensor_scalar_mul(uni[:], uex[:], urs[:])
        ps_u = ppool.tile([P, NL], F32, tag="u_ps", space="PSUM")
        nc.tensor.matmul(ps_u[:], ones_f32[:1, :P], uni[:], start=True, stop=True)
        uni128 = spool.tile([P, NL], F32, tag="uni128", name="uni128")
        nc.vector.tensor_copy(out=uni128[:], in_=ps_u[:])
        out_flat = dr["out"].rearrange("b s c -> (b s) c")
        prefills = []
        for c in range(TC):
            dma = nc.sync.dma_start(out_flat[c * P : (c + 1) * P, :], uni128[:])
            prefills.append(dma.ins)

        for c in range(TC):
            b = c // (S // P)
            ps_lg = ppool.tile([P, NL], F32, tag="lg_ps", space="PSUM")
            for kt in range(KT):
                nc.tensor.matmul(ps_lg[:], _f(x[:, kt, c * P : (c + 1) * P]),
                                 clsw[:, kt], start=(kt == 0), stop=False)
            nc.tensor.matmul(ps_lg[:], ones_f32[:1, :P], clsb[:],
                             start=False, stop=True)
            negmax = spool.tile([P, 1], F32, tag="negmax", name="negmax")
            nc.vector.reduce_max(out=negmax[:], in_=ps_lg[:], negate=True,
                                 axis=mybir.AxisListType.X)
            probs = spool.tile([P, NL], F32, tag="probs", name="probs")
            sm = spool.tile([P, 1], F32, tag="sm", name="sm")
            nc.scalar.activation(probs[:], ps_lg[:], AF.Exp, bias=negmax[:],
                                 accum_out=sm[:])
            rs = spool.tile([P, 1], F32, tag="rs", name="rs")
            nc.vector.reciprocal(rs[:], sm[:])
            nc.vector.tensor_scalar_mul(probs[:], probs[:], rs[:])

            # cumsum of valid over the sequence, sliced to this chunk
            cc = c % (S // P)
            ps_cs = ppool.tile([P, 1], F32, tag="cs_ps", space="PSUM")
            lhs_ks = [ones_f32[:, :P] if ks < cc else ltri
                      for ks in range(cc + 1)]
            for ks, lhs in enumerate(lhs_ks):
                nc.tensor.matmul(ps_cs[:], lhs[:],
                                 valid_f[:, b * 2 + ks : b * 2 + ks + 1],
                                 start=(ks == 0), stop=(ks == len(lhs_ks) - 1))
            # dest = valid ? b*S + csum - 1 : BIG
            dest_f = spool.tile([P, 1], F32, tag="dest_f", name="dest_f")
            nc.vector.tensor_scalar_add(dest_f[:], ps_cs[:], float(b * S - 1 - BIG))
            nc.vector.tensor_tensor(out=dest_f[:], in0=dest_f[:],
                                    in1=valid_f[:, c : c + 1], op=ALU.mult)
            nc.vector.tensor_scalar_add(dest_f[:], dest_f[:], float(BIG))
            dest_i = spool.tile([P, 1], I32, tag="dest_i", name="dest_i")
            nc.vector.tensor_copy(out=dest_i[:], in_=dest_f[:])

            scat = nc.gpsimd.indirect_dma_start(
                out=out_flat[:, :],
                out_offset=bass.IndirectOffsetOnAxis(ap=dest_i[:, :1], axis=0),
                in_=probs[:],
                in_offset=None,
                bounds_check=T - 1, oob_is_err=False,
            )
            for pf in prefills:
                add_dep_helper(scat.ins, pf,
                               reason="scatter after uniform prefill")


_NC_CACHE = {}


def _get_nc(repeat=1, n_layers=L):
    key = (repeat, n_layers)
    if key not in _NC_CACHE:
        _NC_CACHE[key] = build_nc(repeat=repeat, n_layers=n_layers)
    return _NC_CACHE[key]


def make_in_maps(inputs):
    per_seq = {}
    for name in ("input_word_ids", "input_mask", "input_type_ids", "valid_mask"):
        per_seq[name] = np.ascontiguousarray(np.asarray(inputs[name]))
    shared = {
        k: np.ascontiguousarray(np.asarray(v))
        for k, v in inputs.items()
        if k not in per_seq
    }
    in_maps = []
    for c in range(NC):
        m = dict(shared)
        for name, arr in per_seq.items():
            m[name] = np.ascontiguousarray(arr[c * BL : (c + 1) * BL])
        in_maps.append(m)
    return in_maps


def kernel(**inputs):
    nc = _get_nc()
    in_maps = make_in_maps(inputs)
    res = bass_utils.run_bass_kernel_spmd(nc, in_maps, list(range(NC)))
    out = np.concatenate([res.results[c]["out"] for c in range(NC)], axis=0)
    return out.astype(np.float32)



# revision 37
# speedup vs baseline: 1.1966x; 1.1189x over previous
"""BERT-NER (12-layer BERT-base + token compaction + classifier) on 8 TRN2 cores.

Data-parallel over batch: 16 sequences -> 2 per core. Weights replicated.
Activations are kept feature-major (xT: [768 partitions(6 tiles), 512 tokens])
so every x@W matmul uses the stored W[in,out] directly as lhsT.
All matmuls run in float32r (full PE rate for N>=256), storage stays fp32.
LayerNorm stats, softmax row-sums and free-dim biases are folded into small
matmuls (ones vectors) to avoid partition-dim reductions on DVE.
"""

import os
import sys

for _p in ("/opt/trn_rl_repo", "/root/.axon_site/_ro/trn_rl_repo"):
    if os.path.isdir(_p) and _p not in sys.path:
        sys.path.insert(0, _p)

import numpy as np

import concourse.bass as bass
import concourse.mybir as mybir
import concourse.tile as tile
from concourse.tile import add_dep_helper
from concourse import bacc, bass_utils

F32 = mybir.dt.float32
F32R = mybir.dt.float32r
I32 = mybir.dt.int32
AF = mybir.ActivationFunctionType
ALU = mybir.AluOpType

B, S, H, L, A, V, NL = 16, 256, 768, 12, 12, 30522, 9
DH = H // A  # 64
FF = 4 * H  # 3072
NC = 8  # cores
BL = B // NC  # 2 sequences per core
T = BL * S  # 512 tokens per core
KT = H // 128  # 6 k-tiles of the hidden dim
TC = T // 128  # 4 token chunks
BIG = 1_000_000  # OOB dump index for compaction scatter
EPS = 1e-12
ISCALE = 1.0 / float(np.sqrt(DH))

P = 128


def _r(ap):
    """View an AP as float32r (bit-identical, PE reduced precision)."""
    return ap.bitcast(F32R)


def _f(ap):
    """View an f32r AP as plain float32 for DVE/ACT reads."""
    return ap.bitcast(F32)


def build_nc(repeat=1, n_layers=L):
    nc = bacc.Bacc("TRN2", target_bir_lowering=False, debug=False)

    d_ids = nc.dram_tensor("input_word_ids", [BL, S], I32, kind="ExternalInput")
    d_mask = nc.dram_tensor("input_mask", [BL, S], I32, kind="ExternalInput")
    d_type = nc.dram_tensor("input_type_ids", [BL, S], I32, kind="ExternalInput")
    d_valid = nc.dram_tensor("valid_mask", [BL, S], I32, kind="ExternalInput")
    d_wemb = nc.dram_tensor("word_emb", [V, H], F32, kind="ExternalInput")
    d_pemb = nc.dram_tensor("pos_emb", [S, H], F32, kind="ExternalInput")
    d_temb = nc.dram_tensor("type_emb", [2, H], F32, kind="ExternalInput")
    d_elng = nc.dram_tensor("emb_ln_g", [H], F32, kind="ExternalInput")
    d_elnb = nc.dram_tensor("emb_ln_b", [H], F32, kind="ExternalInput")
    d_Wq = nc.dram_tensor("Wq", [L, H, H], F32, kind="ExternalInput")
    d_bq = nc.dram_tensor("bq", [L, H], F32, kind="ExternalInput")
    d_Wk = nc.dram_tensor("Wk", [L, H, H], F32, kind="ExternalInput")
    d_bk = nc.dram_tensor("bk", [L, H], F32, kind="ExternalInput")
    d_Wv = nc.dram_tensor("Wv", [L, H, H], F32, kind="ExternalInput")
    d_bv = nc.dram_tensor("bv", [L, H], F32, kind="ExternalInput")
    d_Wo = nc.dram_tensor("Wo", [L, H, H], F32, kind="ExternalInput")
    d_bo = nc.dram_tensor("bo", [L, H], F32, kind="ExternalInput")
    d_alg = nc.dram_tensor("attn_ln_g", [L, H], F32, kind="ExternalInput")
    d_alb = nc.dram_tensor("attn_ln_b", [L, H], F32, kind="ExternalInput")
    d_W1 = nc.dram_tensor("W1", [L, H, FF], F32, kind="ExternalInput")
    d_b1 = nc.dram_tensor("b1", [L, FF], F32, kind="ExternalInput")
    d_W2 = nc.dram_tensor("W2", [L, FF, H], F32, kind="ExternalInput")
    d_b2 = nc.dram_tensor("b2", [L, H], F32, kind="ExternalInput")
    d_flg = nc.dram_tensor("ffn_ln_g", [L, H], F32, kind="ExternalInput")
    d_flb = nc.dram_tensor("ffn_ln_b", [L, H], F32, kind="ExternalInput")
    d_clsW = nc.dram_tensor("cls_W", [H, NL], F32, kind="ExternalInput")
    d_clsb = nc.dram_tensor("cls_b", [NL], F32, kind="ExternalInput")
    d_out = nc.dram_tensor("out", [BL, S, NL], F32, kind="ExternalOutput")

    dr = dict(
        ids=d_ids, mask=d_mask, type=d_type, valid=d_valid, wemb=d_wemb,
        pemb=d_pemb, temb=d_temb, elng=d_elng, elnb=d_elnb,
        Wq=d_Wq, bq=d_bq, Wk=d_Wk, bk=d_bk, Wv=d_Wv, bv=d_bv, Wo=d_Wo, bo=d_bo,
        alg=d_alg, alb=d_alb, W1=d_W1, b1=d_b1, W2=d_W2, b2=d_b2,
        flg=d_flg, flb=d_flb, clsW=d_clsW, clsb=d_clsb, out=d_out,
    )

    with nc.allow_low_precision(reason="float32r matmul pipeline"), tile.TileContext(
        nc
    ) as tc:
        with (
            tc.tile_pool(name="const", bufs=1) as cpool,
            tc.tile_pool(name="main", bufs=1) as mpool,
            tc.tile_pool(name="wts", bufs=6) as wpool,
            tc.tile_pool(name="w2p", bufs=8) as w2pool,
            tc.tile_pool(name="hrows", bufs=2) as rpool,
            tc.tile_pool(name="hbuf", bufs=3) as hpool,
            tc.tile_pool(name="ebuf", bufs=4) as epool,
            tc.tile_pool(name="small", bufs=2) as spool,
        ):
            pools = dict(c=cpool, m=mpool, w=wpool, w2=w2pool, r=rpool,
                         h=hpool, e=epool, s=spool)
            # ---- constants (device-generated) ----
            ident = cpool.tile([P, P], F32, tag="ident")
            nc.gpsimd.memset(ident[:], 0.0)
            nc.gpsimd.affine_select(
                out=ident[:], in_=ident[:], compare_op=ALU.not_equal, fill=1.0,
                base=0, pattern=[[-1, P]], channel_multiplier=1,
            )
            ones_f32 = cpool.tile([P, 512], F32, tag="ones_f32")
            nc.gpsimd.memset(ones_f32[:], 1.0)
            ones_row = cpool.tile([1, 512], BF16, tag="ones_row")
            nc.vector.tensor_copy(out=ones_row[:], in_=ones_f32[:1, :])
            ones128 = cpool.tile([P, P], F32R, tag="ones128")
            nc.vector.tensor_copy(out=ones128[:], in_=ones_f32[:, :P])
            ones256r = cpool.tile([P, S], F32R, tag="ones256r")
            nc.vector.tensor_copy(out=ones256r[:], in_=ones_f32[:, :S])
            # onehot12[:, h, :]: [128, 12] lhsT whose column h is all-ones --
            # rowsum matmul lands head h's softmax denominator on partition h.
            # (built in f32, then DVE-copied to f32r: fp32r matmul operands
            # must come from a rounding producer, the BIR verifier checks.)
            onehot12f = cpool.tile([P, A, A], F32, tag="cscratch")
            nc.gpsimd.memset(onehot12f[:], 1.0)
            nc.gpsimd.affine_select(
                out=onehot12f[:], in_=onehot12f[:], compare_op=ALU.is_equal,
                fill=0.0, base=0, pattern=[[-1, A], [1, A]],
                channel_multiplier=0,
            )
            onehot12 = cpool.tile([P, A, A], BF16, tag="onehot12")
            nc.vector.tensor_copy(out=onehot12[:], in_=onehot12f[:])
            zero12f = cpool.tile([P, A], F32, tag="cscratch")
            nc.gpsimd.memset(zero12f[:], 0.0)
            zero12 = cpool.tile([P, A], F32R, tag="zero12")
            nc.vector.tensor_copy(out=zero12[:], in_=zero12f[:])
            # dsel[:, ft]: [12, 128] lhsT; column m is one-hot at row
            # 2*ft + (m >= 64) -- broadcasts the two per-head denominator rows
            # of tile ft across its 2x64 partitions.
            dself = cpool.tile([A, KT, 2, DH], F32, tag="cscratch")
            nc.gpsimd.memset(dself[:], 1.0)
            nc.gpsimd.affine_select(
                out=dself[:], in_=dself[:], compare_op=ALU.is_equal, fill=0.0,
                base=0, pattern=[[-2, KT], [-1, 2], [0, DH]],
                channel_multiplier=1,
            )
            dsel = cpool.tile([A, KT, 2, DH], F32R, tag="dsel")
            nc.vector.tensor_copy(out=dsel[:], in_=dself[:])
            # lower-triangular-inclusive block: tri[p, t] = 1 if p <= t
            ltri_f = cpool.tile([P, P], F32, tag="ltri_f")
            nc.gpsimd.memset(ltri_f[:], 1.0)
            nc.gpsimd.affine_select(
                out=ltri_f[:], in_=ltri_f[:], compare_op=ALU.is_ge, fill=0.0,
                base=0, pattern=[[1, P]], channel_multiplier=-1,
            )
            c_eps = cpool.tile([P, 1], F32, tag="c_eps")
            nc.gpsimd.memset(c_eps[:], EPS)
            consts = dict(ident=ident, ones_row=ones_row, ltri=ltri_f,
                          c_eps=c_eps, ones_f32=ones_f32, ones128=ones128,
                          ones256r=ones256r, onehot12=onehot12,
                          zero12=zero12, dsel=dsel)

            def body():
                emit_body(nc, tc, pools, consts, dr, n_layers)

            if repeat == 1:
                body()
            else:
                with tc.For_i(0, repeat, 1):
                    body()

    nc.compile()
    return nc


BF16 = mybir.dt.bfloat16


def _load_w_full(nc, wpool, d_slice):
    """Load a [H, 768] DRAM slice as SBUF [128, KT, 768] bf16 (k-tiles on
    partitions). fp32->bf16 cast happens inline in the SWDGE DMA; bf16
    halves the SBUF footprint (deeper prefetch) and enables FWL fast
    weight loads on the PE."""
    w = wpool.tile([P, KT, H], BF16, tag="w_big", name="w_big")
    src = d_slice.rearrange("(kt p) c -> p kt c", p=P)
    nc.gpsimd.dma_start(w[:, 0:3], src[:, 0:3])
    nc.gpsimd.dma_start(w[:, 3:6], src[:, 3:6])
    return w


def _bias_col(nc, spool, d_vec, tag):
    """Load [H] DRAM vector as [128, KT] (col m = slice m*128:(m+1)*128)."""
    t = spool.tile([P, KT], F32, tag=tag, name=tag)
    nc.sync.dma_start(t[:], d_vec.rearrange("(kt p) -> p kt", p=P))
    return t


def _bias_row(nc, rpool, d_vec, tag="brow", dtype=None):
    """Load a DRAM vector [N<=768] as a single-partition row [1, N].
    Default bf16 (cast in the SWDGE DMA) to match the bf16 matmul path."""
    n = d_vec.shape[0]
    if dtype is None:
        t = rpool.tile([1, n], BF16, tag=tag, name=tag)
        nc.gpsimd.dma_start(t[:], d_vec[None, :])
    else:
        t = rpool.tile([1, n], dtype, tag=tag, name=tag)
        nc.sync.dma_start(t[:], d_vec[None, :].bitcast(dtype))
    return t


def emit_ln_half(nc, ppool, mpool, spool, y, hb, g_col, b_col, out, consts,
                 ltag="pp"):
    """Feature-major layernorm over token half hb (columns hb*S..hb*S+S).
    Stat matmuls use an all-ones [128,128] lhsT so per-token sums land
    broadcast across 128 partitions; sum and sum-of-squares share one PSUM
    bank (two regions), ordered by an explicit dep after the bank clear."""
    ones128 = consts["ones128"]
    o = hb * S
    ln = ppool.tile([P, 2, S], F32, tag=ltag, bufs=2, space="PSUM",
                    name="ln_ps")
    mm_clear = None
    for kt in range(KT):
        sq = mpool.tile([P, S], F32R, tag="ln_sq", bufs=2, name="sq")
        nc.scalar.activation(sq[:], _f(y[:, kt, o : o + S]), AF.Square)
        mm1 = nc.tensor.matmul(ln[:, 0], ones128[:], y[:, kt, o : o + S],
                               start=(kt == 0), stop=(kt == KT - 1),
                               skip_group_check=True)
        mm2 = nc.tensor.matmul(ln[:, 1], ones128[:], sq[:],
                               start=False, stop=(kt == KT - 1),
                               skip_group_check=True)
        if kt == 0:
            add_dep_helper(mm2.ins, mm1.ins, reason="sumsq after bank clear")
    mean = spool.tile([P, S], F32, tag="ln_mean", bufs=2, name="ln_mean")
    nc.vector.tensor_scalar_mul(mean[:], ln[:, 0], 1.0 / H)
    m2 = spool.tile([P, S], F32, tag="ln_m2", bufs=1, name="ln_m2")
    nc.vector.tensor_tensor(out=m2[:], in0=mean[:], in1=mean[:], op=ALU.mult)
    var = spool.tile([P, S], F32, tag="ln_var", bufs=1, name="ln_var")
    nc.vector.scalar_tensor_tensor(out=var[:], in0=ln[:, 1], scalar=1.0 / H,
                                   in1=m2[:], op0=ALU.mult, op1=ALU.subtract)
    std = spool.tile([P, S], F32, tag="ln_std", bufs=1, name="ln_std")
    sqrt_ins = nc.scalar.activation(std[:], var[:], AF.Sqrt,
                                    bias=consts["c_eps"][:])
    rstd = spool.tile([P, S], F32, tag="ln_rstd", bufs=2, name="ln_rstd")
    nc.vector.reciprocal(rstd[:], std[:])
    for kt in range(KT):
        tmp = mpool.tile([P, S], F32, tag="ln_tmp", bufs=2, name="tmp")
        nc.vector.tensor_tensor(out=tmp[:], in0=_f(y[:, kt, o : o + S]),
                                in1=mean[:], op=ALU.subtract)
        nc.vector.tensor_tensor(out=tmp[:], in0=tmp[:], in1=rstd[:],
                                op=ALU.mult)
        nc.scalar.activation(out[:, kt, o : o + S], tmp[:], AF.Identity,
                             scale=g_col[:, kt : kt + 1],
                             bias=b_col[:, kt : kt + 1])
    return sqrt_ins


def emit_body(nc, tc, pools, consts, dr, n_layers):
    cpool, mpool, wpool, w2pool = (
        pools["c"], pools["m"], pools["w"], pools["w2"])
    rpool, hpool, epool, spool = (
        pools["r"], pools["h"], pools["e"], pools["s"])
    ident, ones_row, ltri = (
        consts["ident"], consts["ones_row"], consts["ltri"])
    ones_f32 = consts["ones_f32"]
    onehot12, zero12, dsel = (
        consts["onehot12"], consts["zero12"], consts["dsel"])
    ones256r = consts["ones256r"]

    ids_flat = dr["ids"].rearrange("b s -> (b s)")
    type_flat = dr["type"].rearrange("b s -> (b s)")
    mask_flat = dr["mask"].rearrange("b s -> (b s)")
    valid_flat = dr["valid"].rearrange("b s -> (b s)")

    # amask[:, c]: 0 where mask==1 else -10000 ; valid_f: valid mask as f32r
    amask = cpool.tile([P, TC], F32, tag="amask", name="amask")
    valid_f = cpool.tile([P, TC], F32, tag="valid_f", name="valid_f")

    # ============ embeddings (token-major), transpose, LN ============
    xtok = mpool.tile([P, TC, H], F32, tag="bigA", name="xtok")
    for c in range(TC):
        idt = spool.tile([P, 1], I32, tag="idt", name="idt")
        nc.sync.dma_start(idt[:], ids_flat[c * P : (c + 1) * P, None])
        nc.gpsimd.indirect_dma_start(
            out=xtok[:, c], out_offset=None, in_=dr["wemb"][:, :],
            in_offset=bass.IndirectOffsetOnAxis(ap=idt[:, :1], axis=0),
        )
        tyt = spool.tile([P, 1], I32, tag="tyt", name="tyt")
        nc.sync.dma_start(tyt[:], type_flat[c * P : (c + 1) * P, None])
        # accumulate type + position embeddings in-flight (SWDGE CCE add)
        nc.gpsimd.indirect_dma_start(
            out=xtok[:, c], out_offset=None, in_=dr["temb"][:, :],
            in_offset=bass.IndirectOffsetOnAxis(ap=tyt[:, :1], axis=0),
            compute_op=ALU.add,
        )
        cc = c % (S // P)
        nc.gpsimd.dma_start(xtok[:, c], dr["pemb"][cc * P : (cc + 1) * P, :],
                            accum_op=ALU.add)

        mi = spool.tile([P, 1], I32, tag="mi", name="mi")
        nc.sync.dma_start(mi[:], mask_flat[c * P : (c + 1) * P, None])
        mf = spool.tile([P, 1], F32, tag="mf", name="mf")
        nc.vector.tensor_copy(out=mf[:], in_=mi[:])
        nc.scalar.activation(amask[:, c : c + 1], mf[:], AF.Copy,
                             scale=10000.0, bias=-10000.0)
        vi = spool.tile([P, 1], I32, tag="vi", name="vi")
        nc.sync.dma_start(vi[:], valid_flat[c * P : (c + 1) * P, None])
        nc.vector.tensor_copy(out=valid_f[:, c : c + 1], in_=vi[:])

    # transpose to feature-major
    xe = mpool.tile([P, KT, 512], F32R, tag="bigB", name="xe")
    x = mpool.tile([P, KT, 512], BF16, tag="x_cur", name="x0")
    with tc.tile_pool(name="embtr", bufs=1, space="PSUM") as ppool:
        for kt in range(KT):
            for c in range(TC):
                ps_t = ppool.tile([P, P], F32, tag="tr", bufs=2, space="PSUM")
                nc.tensor.transpose(
                    out=ps_t[:], in_=xtok[:, c, kt * P : (kt + 1) * P],
                    identity=ident[:])
                nc.vector.tensor_copy(out=xe[:, kt, c * P : (c + 1) * P],
                                      in_=ps_t[:])
        eg = _bias_col(nc, spool, dr["elng"][:], "eg")
        eb = _bias_col(nc, spool, dr["elnb"][:], "eb")
        for hb in range(BL):
            emit_ln_half(nc, ppool, mpool, spool, xe, hb, eg, eb, x, consts)

    # ============ transformer layers ============
    for l in range(n_layers):
        # ---- Q, K projections (feature-major out) ----
        qT = mpool.tile([P, KT, 512], BF16, tag="bigA", name="qT")
        kT = mpool.tile([P, KT, 512], BF16, tag="bigB", name="kT")
        vsb = mpool.tile([P, TC, A, DH], BF16, tag="vsb", name="vsb")
        craw = mpool.tile([P, KT, 512], F32, tag="bigC", name="craw")
        ctxT = mpool.tile([P, KT, 512], BF16, tag="bigD", name="ctxT")
        y1 = mpool.tile([P, KT, 512], F32R, tag="bigE", name="y1")
        x2 = mpool.tile([P, KT, 512], BF16, tag="bigF", name="x2")

        # ---- phase 1: QKV projections + attention + Wo + LN1, pipelined by
        # sequence half (the two sequences are independent until the FFN's
        # shared weight streaming). All matmuls are N=256 (full f32r rate).
        with tc.tile_pool(name=f"ph1_{l}", bufs=1, space="PSUM") as ppool:
            bq_c = _bias_col(nc, spool, dr["bq"][l], "b_q")
            bk_c = _bias_col(nc, spool, dr["bk"][l], "b_k")
            bvr = _bias_row(nc, rpool, dr["bv"][l])
            wq = _load_w_full(nc, wpool, dr["Wq"][l])
            wk = _load_w_full(nc, wpool, dr["Wk"][l])
            wv = _load_w_full(nc, wpool, dr["Wv"][l])
            for hb in range(BL):
                o = hb * S
                for w, bc, dst in ((wq, bq_c, qT), (wk, bk_c, kT)):
                    for m in range(KT):
                        ps = ppool.tile([P, S], F32, tag="pp", bufs=2,
                                        space="PSUM")
                        for kt in range(KT):
                            nc.tensor.matmul(ps[:], w[:, kt, m * P : (m + 1) * P],
                                             x[:, kt, o : o + S],
                                             start=(kt == 0), stop=(kt == KT - 1))
                        nc.scalar.activation(dst[:, m, o : o + S], ps[:],
                                             AF.Identity, bias=bc[:, m : m + 1])
                # V (token-major) for this half's two chunks
                for c in (2 * hb, 2 * hb + 1):
                    for hh in range(2):
                        ps = ppool.tile([P, 384], F32, tag="pp", bufs=2,
                                        space="PSUM")
                        for kt in range(KT):
                            nc.tensor.matmul(
                                ps[:], x[:, kt, c * P : (c + 1) * P],
                                wv[:, kt, hh * 384 : (hh + 1) * 384],
                                start=(kt == 0), stop=False)
                        nc.tensor.matmul(ps[:], ones_row[:1, :P],
                                         bvr[:1, hh * 384 : (hh + 1) * 384],
                                         start=False, stop=True)
                        nc.vector.tensor_copy(
                            out=vsb[:, c, hh * 6 : (hh + 1) * 6, :],
                            in_=ps[:].rearrange("p (h d) -> p h d", d=DH))
            wo = _load_w_full(nc, wpool, dr["Wo"][l])
            bo_c = _bias_col(nc, spool, dr["bo"][l], "bo_c")
            ag = _bias_col(nc, spool, dr["alg"][l], "ag")
            ab = _bias_col(nc, spool, dr["alb"][l], "ab")
            # attention per (half, head): raw context + per-head denominators
            # accumulated in a [12, 256] PSUM bank (row h = head h) via
            # one-hot lhsT columns; normalization happens once per half.
            ln1_sqrt0 = None
            last_exp = [None]
            for b in range(BL):
                o = b * S
                ps_D = ppool.tile([A, S], F32, tag="d_ps", bufs=2,
                                  space="PSUM")
                nc.tensor.matmul(ps_D[:], zero12[:], ones256r[:],
                                 start=True, stop=False, skip_group_check=True)
                for h in range(A):
                    ft, fo = h // 2, (h % 2) * DH
                    esb = []
                    for kc in range(2):
                        ps_s = ppool.tile([P, S], F32, tag="s_ps", bufs=3,
                                          space="PSUM")
                        nc.tensor.matmul(
                            ps_s[:],
                            kT[fo : fo + DH, ft, o + kc * P : o + (kc + 1) * P],
                            qT[fo : fo + DH, ft, o : o + S],
                            start=True, stop=True)
                        e = epool.tile([P, S], BF16, tag="e_sb", bufs=10,
                                       name="e_sb")
                        last_exp[0] = nc.scalar.activation(
                            e[:], ps_s[:], AF.Exp, scale=ISCALE,
                            bias=amask[:, b * 2 + kc : b * 2 + kc + 1])
                        esb.append(e)
                    ps_c = ppool.tile([DH, S], F32, tag="c_ps", bufs=1,
                                      space="PSUM")
                    for kc in range(2):
                        nc.tensor.matmul(ps_c[:], vsb[:, b * 2 + kc, h],
                                         esb[kc][:], start=(kc == 0),
                                         stop=(kc == 1))
                    nc.vector.tensor_copy(
                        out=craw[fo : fo + DH, ft, o : o + S], in_=ps_c[:])
                    for kc in range(2):
                        nc.tensor.matmul(ps_D[:], onehot12[:, h, :],
                                         esb[kc][:], start=False,
                                         stop=(h == A - 1 and kc == 1),
                                         skip_group_check=True)
                drec = spool.tile([A, S], F32R, tag="drec", bufs=2,
                                  name="drec")
                nc.vector.reciprocal(drec[:], ps_D[:])
                for ft in range(KT):
                    ps_bc = ppool.tile([P, S], F32, tag="pp", bufs=2,
                                       space="PSUM")
                    nc.tensor.matmul(ps_bc[:], dsel[:, ft], drec[:],
                                     start=True, stop=True)
                    nc.vector.tensor_tensor(
                        out=ctxT[:, ft, o : o + S], in0=ps_bc[:],
                        in1=craw[:, ft, o : o + S], op=ALU.mult)
                # Wo + residual (bias folded into DVE) + LN1, this half
                for m in range(KT):
                    ps = ppool.tile([P, S], F32, tag="pp", bufs=2,
                                    space="PSUM")
                    for kt in range(KT):
                        nc.tensor.matmul(ps[:], wo[:, kt, m * P : (m + 1) * P],
                                         ctxT[:, kt, o : o + S],
                                         start=(kt == 0), stop=(kt == KT - 1))
                    nc.vector.scalar_tensor_tensor(
                        out=y1[:, m, o : o + S], in0=ps[:],
                        scalar=bo_c[:, m : m + 1], in1=x[:, m, o : o + S],
                        op0=ALU.add, op1=ALU.add)
                s_ins = emit_ln_half(nc, ppool, mpool, spool, y1, b, ag, ab,
                                     x2, consts)
                if b == 0:
                    ln1_sqrt0 = s_ins
            # keep LN1(b0)'s Sqrt (foreign ACT table set) after the second
            # half's exps so the exp->sqrt->exp table reload ping-pong
            # collapses to one switch per layer
            if ln1_sqrt0 is not None and last_exp[0] is not None:
                add_dep_helper(ln1_sqrt0.ins, last_exp[0].ins,
                               reason="defer sqrt past exps (act table)")

        # ---- phase 2: FFN (y2 in 6 persistent PSUM banks, halves
        # interleaved per chunk so each W1/W2 chunk is loaded once) + LN2 ----
        y3 = mpool.tile([P, KT, 512], F32R, tag="bigE", name="y3")
        xn = mpool.tile([P, KT, 512], BF16, tag="x_cur", name="xn")
        with (
            tc.tile_pool(name=f"ffa{l}", bufs=1, space="PSUM") as papool,
            tc.tile_pool(name=f"ffh{l}", bufs=1, space="PSUM") as fpool,
        ):
            b2_c = _bias_col(nc, spool, dr["b2"][l], "b2_c")
            # b1 in feature-major column layout [128, 24]: folded into Gelu
            b1_c = spool.tile([P, 4 * KT], F32, tag="b1_c", name="b1_c")
            nc.sync.dma_start(b1_c[:], dr["b1"][l].rearrange("(kt p) -> p kt", p=P))
            fg = _bias_col(nc, spool, dr["flg"][l], "fg")
            fb = _bias_col(nc, spool, dr["flb"][l], "fb")
            ps_y = [papool.tile([P, 512], F32, tag=f"acc{m}",
                                name=f"ps_y{l}_{m}", space="PSUM")
                    for m in range(KT)]
            first_mm = {}
            for q4 in range(4):
                w1 = _load_w_full(nc, wpool, dr["W1"][l][:, q4 * H : (q4 + 1) * H])
                for cc in range(KT):
                    c = q4 * KT + cc
                    w2 = w2pool.tile([P, H], BF16, tag="w2c", name="w2")
                    nc.gpsimd.dma_start(w2[:],
                                        dr["W2"][l][c * P : (c + 1) * P, :])
                    for hb in range(BL):
                        o = hb * S
                        ps_h = fpool.tile([P, S], F32, tag="h_ps", bufs=2,
                                          space="PSUM")
                        for kt in range(KT):
                            nc.tensor.matmul(ps_h[:],
                                             w1[:, kt, cc * P : (cc + 1) * P],
                                             x2[:, kt, o : o + S],
                                             start=(kt == 0), stop=(kt == KT - 1))
                        hsb = hpool.tile([P, S], BF16, tag="h_sb", bufs=6,
                                         name="hsb")
                        nc.scalar.activation(hsb[:], ps_h[:],
                                             AF.Gelu_apprx_tanh,
                                             bias=b1_c[:, c : c + 1])
                        for m in range(KT):
                            mm = nc.tensor.matmul(
                                ps_y[m][:, o : o + S],
                                w2[:, m * P : (m + 1) * P], hsb[:],
                                start=(c == 0 and hb == 0),
                                stop=(c == 4 * KT - 1 and hb == 1),
                                skip_group_check=True)
                            if c == 0:
                                if hb == 0:
                                    first_mm[m] = mm
                                else:
                                    add_dep_helper(mm.ins, first_mm[m].ins,
                                                   reason="acc after bank clear")
            for hb in range(BL):
                o = hb * S
                for m in range(KT):
                    nc.vector.scalar_tensor_tensor(
                        out=y3[:, m, o : o + S], in0=ps_y[m][:, o : o + S],
                        scalar=b2_c[:, m : m + 1], in1=x2[:, m, o : o + S],
                        op0=ALU.add, op1=ALU.add)
                emit_ln_half(nc, fpool, mpool, spool, y3, hb, fg, fb, xn,
                             consts, ltag="h_ps")
        x = xn

    # ============ classifier head + softmax + compaction ============
    with tc.tile_pool(name="head", bufs=2, space="PSUM") as ppool:
        clsw = spool.tile([P, KT, NL], BF16, tag="clsw", name="clsw")
        nc.gpsimd.dma_start(clsw[:], dr["clsW"].rearrange("(kt p) c -> p kt c", p=P))
        clsb = _bias_row(nc, rpool, dr["clsb"][:], tag="clsb", dtype=F32)

        # uniform pad row: softmax(cls_b), broadcast to 128 partitions
        nmx = spool.tile([1, 1], F32, tag="nmx", name="nmx")
        nc.vector.reduce_max(out=nmx[:], in_=clsb[:], negate=True,
                             axis=mybir.AxisListType.X)
        usum = spool.tile([1, 1], F32, tag="usum", name="usum")
        uex = spool.tile([1, NL], F32, tag="uex", name="uex")
        nc.scalar.activation(uex[:], clsb[:], AF.Exp, bias=nmx[:],
                             accum_out=usum[:])
        urs = spool.tile([1, 1], F32, tag="urs", name="urs")
        nc.vector.reciprocal(urs[:], usum[:])
        uni = spool.tile([1, NL], F32, tag="uni", name="uni")
        nc.vector.tensor_scalar_mul(uni[:], uex[:], urs[:])
        ps_u = ppool.tile([P, NL], F32, tag="u_ps", space="PSUM")
        nc.tensor.matmul(ps_u[:], ones_f32[:1, :P], uni[:], start=True, stop=True)
        uni128 = spool.tile([P, NL], F32, tag="uni128", name="uni128")
        nc.vector.tensor_copy(out=uni128[:], in_=ps_u[:])
        out_flat = dr["out"].rearrange("b s c -> (b s) c")
        prefills = []
        for c in range(TC):
            dma = nc.sync.dma_start(out_flat[c * P : (c + 1) * P, :], uni128[:])
            prefills.append(dma.ins)

        for c in range(TC):
            b = c // (S // P)
            ps_lg = ppool.tile([P, NL], F32, tag="lg_ps", space="PSUM")
            for kt in range(KT):
                nc.tensor.matmul(ps_lg[:], x[:, kt, c * P : (c + 1) * P],
                                 clsw[:, kt], start=(kt == 0), stop=False)
            nc.tensor.matmul(ps_lg[:], ones_f32[:1, :P], clsb[:],
                             start=False, stop=True)
            negmax = spool.tile([P, 1], F32, tag="negmax", name="negmax")
            nc.vector.reduce_max(out=negmax[:], in_=ps_lg[:], negate=True,
                                 axis=mybir.AxisListType.X)
            probs = spool.tile([P, NL], F32, tag="probs", name="probs")
            sm = spool.tile([P, 1], F32, tag="sm", name="sm")
            nc.scalar.activation(probs[:], ps_lg[:], AF.Exp, bias=negmax[:],
                                 accum_out=sm[:])
            rs = spool.tile([P, 1], F32, tag="rs", name="rs")
            nc.vector.reciprocal(rs[:], sm[:])
            nc.vector.tensor_scalar_mul(probs[:], probs[:], rs[:])

            # cumsum of valid over the sequence, sliced to this chunk
            cc = c % (S // P)
            ps_cs = ppool.tile([P, 1], F32, tag="cs_ps", space="PSUM")
            lhs_ks = [ones_f32[:, :P] if ks < cc else ltri
                      for ks in range(cc + 1)]
            for ks, lhs in enumerate(lhs_ks):
                nc.tensor.matmul(ps_cs[:], lhs[:],
                                 valid_f[:, b * 2 + ks : b * 2 + ks + 1],
                                 start=(ks == 0), stop=(ks == len(lhs_ks) - 1))
            # dest = valid ? b*S + csum - 1 : BIG
            dest_f = spool.tile([P, 1], F32, tag="dest_f", name="dest_f")
            nc.vector.tensor_scalar_add(dest_f[:], ps_cs[:], float(b * S - 1 - BIG))
            nc.vector.tensor_tensor(out=dest_f[:], in0=dest_f[:],
                                    in1=valid_f[:, c : c + 1], op=ALU.mult)
            nc.vector.tensor_scalar_add(dest_f[:], dest_f[:], float(BIG))
            dest_i = spool.tile([P, 1], I32, tag="dest_i", name="dest_i")
            nc.vector.tensor_copy(out=dest_i[:], in_=dest_f[:])

            scat = nc.gpsimd.indirect_dma_start(
                out=out_flat[:, :],
                out_offset=bass.IndirectOffsetOnAxis(ap=dest_i[:, :1], axis=0),
                in_=probs[:],
                in_offset=None,
                bounds_check=T - 1, oob_is_err=False,
            )
            for pf in prefills:
                add_dep_helper(scat.ins, pf,
                               reason="scatter after uniform prefill")


_NC_CACHE = {}


def _get_nc(repeat=1, n_layers=L):
    key = (repeat, n_layers)
    if key not in _NC_CACHE:
        _NC_CACHE[key] = build_nc(repeat=repeat, n_layers=n_layers)
    return _NC_CACHE[key]


def make_in_maps(inputs):
    per_seq = {}
    for name in ("input_word_ids", "input_mask", "input_type_ids", "valid_mask"):
        per_seq[name] = np.ascontiguousarray(np.asarray(inputs[name]))
    shared = {
        k: np.ascontiguousarray(np.asarray(v))
        for k, v in inputs.items()
        if k not in per_seq
    }
    in_maps = []
    for c in range(NC):
        m = dict(shared)
        for name, arr in per_seq.items():
            m[name] = np.ascontiguousarray(arr[c * BL : (c + 1) * BL])
        in_maps.append(m)
    return in_maps


def kernel(**inputs):
    nc = _get_nc()
    in_maps = make_in_maps(inputs)
    res = bass_utils.run_bass_kernel_spmd(nc, in_maps, list(range(NC)))
    out = np.concatenate([res.results[c]["out"] for c in range(NC)], axis=0)
    return out.astype(np.float32)

